# revision 1
# baseline (speedup 1.0000x reference)
"""Trainium2 Bass kernel for nn_BinaryTTN (batch 2048, 12-layer binary tree
tensor network), data-parallel across 8 NeuronCores.

Structure (per core, n=256 samples):
  * Layers 0+1 folded on host into layer-2 weights; the 16 z4 monomials per
    2x2 patch are pure input packing and are computed on host (fp32, cast to
    fp16) and DMA'd in, in the layout the layer-2 matmuls consume.
  * Each tree location (li=2..10): t = Wfold^T l (PE, K=16 row-tiled, rhs is
    the left child's compact [16,n] output), then an elementwise multiply by
    the right child's REP-form [128,n] (8x replicated rows), split across
    DVE/ACT/GPSIMD paths, then a PE reduction over j:
      - locations consumed as LEFT by their parent reduce via a col-tiled
        [K=128, M=16] matmul into a shared compact PSUM bank (4 locs/bank),
        evacuated once per 4 locs -> compact [16,n] fp16.
      - locations consumed as RIGHT reduce via the M=128 replicating matmul
        (baseline 'red') -> REP form, evacuated per loc.
  * Multiply paths (per pair of locations, statically assigned):
      D: DVE reads t from PSUM fp32 directly (1x mode)
      A: ACT evacuates t to fp16 SBUF, DVE multiplies at 2x
      GD: DVE copies t PSUM->SBUF, GPSIMD multiplies
      GA: ACT evacuates, GPSIMD multiplies
"""
import sys
import numpy as np

sys.path.insert(0, '/opt/trn_rl_repo')

BATCH, EMBED, H0, W0 = 2048, 2, 64, 64
NCORES = 8
NSH = BATCH // NCORES      # 256
NBLK = 8

# mult-path pattern, cycled over loc-pairs: D / A / GD / GA
PATH_PATTERN = ['A', 'D', 'GD', 'A', 'D', 'D', 'A', 'D',
                'GD', 'A', 'D', 'D', 'A', 'D', 'GD', 'A']


def layer_specs():
    out = []
    H, W, ind = H0, W0, EMBED
    for li in range(12):
        bond = 1 if li == 11 else 16
        o = H < W
        h = H // (1 if o else 2)
        w = W // (2 if o else 1)
        out.append((h, w, bond, ind, o))
        H, W, ind = h, w, bond
    return out


SPECS = layer_specs()


def role_of(li, y, x):
    """'l' or 'r': how the parent consumes this loc's output."""
    if li == 11:
        return 'l'
    o_p = SPECS[li + 1][4]
    if o_p:
        return 'l' if x % 2 == 0 else 'r'
    return 'l' if y % 2 == 0 else 'r'


def schedule_blk(blk):
    """Yields (li, [locs...]) pair-batches; four independent y2 rows are
    interleaved so the scheduler always has independent work nearby."""
    for y2 in range(0, 16, 4):
        for c0 in range(0, 4, 2):
            for dy in range(4):
                yield (2, [(y2 + dy, 4 * blk + c0), (y2 + dy, 4 * blk + c0 + 1)])
        for dy in range(4):
            yield (3, [(y2 + dy, 2 * blk), (y2 + dy, 2 * blk + 1)])
        y4 = y2 // 2
        yield (4, [(y4, 2 * blk), (y4, 2 * blk + 1)])
        yield (4, [(y4 + 1, 2 * blk), (y4 + 1, 2 * blk + 1)])
        yield (5, [(y4, blk)])
        yield (5, [(y4 + 1, blk)])
        if y2 == 4:
            yield (6, [(0, blk), (1, blk)])
        elif y2 == 12:
            yield (6, [(2, blk), (3, blk)])


def schedule_tail():
    for li in range(7, 11):
        h, w = SPECS[li][0], SPECS[li][1]
        locs = [(y, x) for y in range(h) for x in range(w)]
        for i0 in range(0, len(locs), 2):
            yield (li, locs[i0:i0 + 2])


def _build_structure():
    """Walks the schedule; assigns per-loc meta:
      role, a (t-MM row group = q of left child / g of z4 slice),
      q+group (l-locs), path (per pair), slab offset per (li, blk, a)."""
    meta = {}
    slab_off = {}
    slabs = {}

    def loc_children(li, y, x):
        orient = SPECS[li][4]
        cl = (li - 1, y, 2 * x) if orient else (li - 1, 2 * y, x)
        cr = (li - 1, y, 2 * x + 1) if orient else (li - 1, 2 * y + 1, x)
        return cl, cr

    # pass 1: emission order + consumer batch index per loc
    batches = []
    for blk in range(NBLK):
        for li, locs in schedule_blk(blk):
            batches.append((li, locs, blk))
        batches.append(None)            # group-flush boundary
    for li, locs in schedule_tail():
        batches.append((li, locs, 0))
    batches.append(None)

    consumed_at = {}
    for bi, b in enumerate(batches):
        if b is None:
            continue
        li, locs, blk = b
        for (y, x) in locs:
            if li >= 3:
                cl, cr = loc_children(li, y, x)
                consumed_at[cl] = bi
                consumed_at[cr] = bi

    # pass 2: consumer-aware group assignment
    state = {'group': 0, 'pair': 0}
    groups = {}
    open_members = []

    def close_group():
        if open_members:
            groups[state['group']] = len(open_members)
            state['group'] += 1
            open_members.clear()

    def emit(li, locs, blk, bi):
        path = PATH_PATTERN[state['pair'] % len(PATH_PATTERN)]
        state['pair'] += 1
        for (y, x) in locs:
            if li == 2:
                a = (y + x) % 4
            else:
                cl, _ = loc_children(li, y, x)
                a = meta[cl]['q']
            off = slab_off.get((li, a), 0)
            slab_off[(li, a)] = off + 256
            if (li, blk, a) not in slabs:
                slabs[(li, blk, a)] = off      # start col for this blk's slab
            m = dict(role=role_of(li, y, x), a=a, blk=blk, path=path,
                     woff=off - slabs[(li, blk, a)])
            if m['role'] == 'l':
                m['q'] = len(open_members)
                m['group'] = state['group']
                open_members.append((li, y, x))
                if len(open_members) == 4:
                    close_group()
            meta[(li, y, x)] = m

    for bi, b in enumerate(batches):
        if b is None:
            close_group()
            continue
        li, locs, blk = b
        emit(li, locs, blk, bi)
        # close if the next batch consumes any open member
        nxt = batches[bi + 1] if bi + 1 < len(batches) else None
        if open_members and nxt is not None:
            if any(consumed_at.get(k, 10 ** 9) <= bi + 1 for k in open_members):
                close_group()

    # sanity: every loc's group must close before its consumer batch
    close_bi = {}
    cnt = {}
    for bi, b in enumerate(batches):
        if b is None:
            continue
        li, locs, blk = b
        for (y, x) in locs:
            m = meta[(li, y, x)]
            if 'group' in m:
                g = m['group']
                cnt[g] = cnt.get(g, 0) + 1
                if cnt[g] == groups[g]:
                    close_bi[g] = bi
    for (li, y, x), m in meta.items():
        if 'group' in m and (li, y, x) in consumed_at:
            assert close_bi[m['group']] < consumed_at[(li, y, x)], \
                f"group {m['group']} closes too late for {(li, y, x)}"

    slabsz = {}
    for (li, blk, a), start in slabs.items():
        end = slab_off[(li, a)]
        nxt = min((s for (li2, b2, a2), s in slabs.items()
                   if li2 == li and a2 == a and s > start), default=end)
        slabsz[(li, blk, a)] = (start, nxt - start)
    shapes = {li: max(slab_off.get((li, a), 0) for a in range(4))
              for li in range(2, 11)}
    return meta, slabsz, shapes, groups


LOC_META, SLABS, WSHAPES, GROUPS = _build_structure()


# ---------------- host packing ----------------
def fold_weights(ws):
    W0m = ws[0].reshape(32, 64, 16, 4)
    T1 = np.einsum('rcbij,rcik,rcjl->rcbkl',
                   ws[1], W0m[:, 0::2], W0m[:, 1::2]).reshape(32, 32, 16, 16)
    Q2 = np.einsum('rcbij,rcik,rcjl->rcbkl', ws[2], T1[0::2], T1[1::2])
    folded = {}
    q = Q2.transpose(0, 1, 3, 2, 4).reshape(16, 32, 16, 2, 8, 16)
    folded[2] = q.transpose(0, 1, 3, 2, 4, 5).reshape(16, 32, 2, 16, 128)
    for li in range(3, 11):
        h, w = SPECS[li][0], SPECS[li][1]
        q = ws[li].transpose(0, 1, 3, 2, 4).reshape(h, w, 16, 2, 8, 16)
        folded[li] = q.transpose(0, 1, 3, 2, 4, 5).reshape(h, w, 2, 16, 128)
    folded[11] = ws[11].transpose(0, 1, 3, 2, 4).reshape(16, 16)
    return folded


def pack_weights(ws):
    folded = fold_weights(ws)
    whbm = {}
    for li in range(2, 11):
        arr = np.zeros((128, WSHAPES[li]), dtype=np.float16)
        for (lli, y, x), m in LOC_META.items():
            if lli != li:
                continue
            a = m['a']
            off = SLABS[(li, m['blk'], a)][0] + m['woff']
            arr[32 * a:32 * a + 16, off:off + 256] = \
                folded[li][y, x].transpose(1, 0, 2).reshape(16, 256)
        whbm[li] = arr
    whbm[11] = folded[11].astype(np.float16)
    return whbm


def build_consts():
    c = {}
    red = np.zeros((128, 256), dtype=np.float16)
    for ch in range(2):
        for bl in range(8):
            b = ch * 8 + bl
            for k in range(16):
                for s in range(8):
                    red[bl * 16 + k, ch * 128 + s * 16 + b] = 1.0
    c['red'] = red
    red16 = np.zeros((128, 32), dtype=np.float16)
    for b in range(16):
        bl, ch = b % 8, b // 8
        for j in range(16):
            red16[bl * 16 + j, ch * 16 + b] = 1.0
    c['red16'] = red16
    repk = np.zeros((128, 128), dtype=np.float16)
    for a in range(4):
        for k in range(16):
            for s in range(8):
                repk[32 * a + k, s * 16 + k] = 1.0
    c['repk'] = repk
    o = np.zeros((128, 4), dtype=np.float16)
    o[0:16, 0] = 1.0
    c['ones16'] = o
    return c


def pack_z4(xsh):
    """xsh [n, 2, 64, 64] fp32 -> z4e/z4o [128, 8*4*4*n], z4r [128, 8*16*4*n].

    z4[n, k=(kl 4, kr 4), r1, c1] = z0L[kl] * z0R[kr], with
    z0[n, (i,j), r, c] = x[n,i,2r,c] * x[n,j,2r+1,c].
    z4e/z4o layout: [32g+k, (blk, rhi, cin, n)] = z4[n, k, rhi*8+2g+par, 4blk+cin]
    z4r layout: [16s+k, (blk, y2, cin, n)] = z4[n, k, 2*y2+1, 4blk+cin]
    """
    n = xsh.shape[0]
    x32 = np.asarray(xsh, dtype=np.float32)
    top = x32[:, :, 0::2, :]
    bot = x32[:, :, 1::2, :]
    z0 = np.einsum('nirc,njrc->nijrc', top, bot).reshape(n, 4, 32, 64)
    z4 = np.einsum('nkrc,nlrc->nklrc', z0[:, :, :, 0::2],
                   z0[:, :, :, 1::2]).reshape(n, 16, 32, 32).astype(np.float16)
    # z4e/z4o: r1 = rhi*8 + g*2 + par
    z4v = z4.reshape(n, 16, 4, 4, 2, 8, 4)  # n, k, rhi, g, par, blk, cin
    arr = z4v[:, :, :, :, 0]  # even rows: n k rhi g blk cin
    arr = arr.transpose(3, 1, 4, 2, 5, 0)  # g k blk rhi cin n
    # rotate partition group by cin so consecutive L2 locs use different
    # PE row groups: slice (y2, c1) lives at group (y2 + c1) % 4
    rot = np.empty_like(arr)
    for cin in range(4):
        rot[:, :, :, :, cin] = np.roll(arr[:, :, :, :, cin], cin, axis=0)
    arr = rot.reshape(4, 16, -1)
    full = np.zeros((4, 32, arr.shape[2]), dtype=np.float16)
    full[:, 0:16] = arr
    z4e = np.ascontiguousarray(full.reshape(128, -1))
    # z4r: bottom rows r1 = 2*y2+1
    zb = z4[:, :, 1::2, :].reshape(n, 16, 16, 8, 4)  # n k y2 blk cin
    zb = zb.transpose(1, 3, 2, 4, 0)  # k blk y2 cin n
    zr = np.broadcast_to(zb[None], (8,) + zb.shape).reshape(128, -1)
    return z4e, np.ascontiguousarray(zr)


# ---------------- device program ----------------
_PROGRAM = None


def build_program(num_devices=NCORES, dbg=None, maxli=11, wq='sync'):
    from contextlib import ExitStack
    import concourse.bass as bass
    import concourse.tile as tile
    from concourse import bacc, mybir

    F16, F32 = mybir.dt.float16, mybir.dt.float32
    n = NSH
    nc = bacc.Bacc("TRN2", target_bir_lowering=False, debug=False,
                   num_devices=num_devices)
    z4e_h = nc.declare_dram_parameter("z4e", [128, 8 * 4 * 4 * n], F16, isOutput=False)
    z4r_h = nc.declare_dram_parameter("z4r", [128, 8 * 16 * 4 * n], F16, isOutput=False)
    wh = {li: nc.declare_dram_parameter(f"w{li}", [128, WSHAPES[li]], F16,
                                        isOutput=False) for li in range(2, 11)}
    wh[11] = nc.declare_dram_parameter("w11", [16, 16], F16, isOutput=False)
    red_h = nc.declare_dram_parameter("red", [128, 256], F16, isOutput=False)
    red16_h = nc.declare_dram_parameter("red16", [128, 32], F16, isOutput=False)
    repk_h = nc.declare_dram_parameter("repk", [128, 128], F16, isOutput=False)
    ones_h = nc.declare_dram_parameter("ones16", [128, 4], F16, isOutput=False)
    out_h = nc.declare_dram_parameter("out", [1, n], F32, isOutput=True)
    dbg_h = (nc.declare_dram_parameter("dbg", [128, n], F16, isOutput=True)
             if dbg is not None else None)

    q11 = LOC_META[(10, 0, 0)]['q'] if (10, 0, 0) in LOC_META else 0

    with tile.TileContext(nc) as tc, ExitStack() as ctx:
        cpool = ctx.enter_context(tc.tile_pool(name="consts", bufs=1))
        red = cpool.tile([128, 256], F16); nc.sync.dma_start(red[:], red_h[:])
        red16 = cpool.tile([128, 32], F16); nc.sync.dma_start(red16[:], red16_h[:])
        repk = cpool.tile([128, 128], F16); nc.sync.dma_start(repk[:], repk_h[:])
        ones16 = cpool.tile([128, 4], F16); nc.sync.dma_start(ones16[:], ones_h[:])
        w11t = cpool.tile([128, 16], F16)
        nc.sync.dma_start(w11t[32 * q11:32 * q11 + 16, :], wh[11][:])

        z4pool = ctx.enter_context(tc.tile_pool(name="z4", bufs=2))
        z4rpool = ctx.enter_context(tc.tile_pool(name="z4r", bufs=1))
        wpool = ctx.enter_context(tc.tile_pool(name="w", bufs=2))
        tpool = ctx.enter_context(tc.tile_pool(name="t", bufs=6))
        mpool = ctx.enter_context(tc.tile_pool(name="m", bufs=6))
        csbpool = ctx.enter_context(tc.tile_pool(name="csb", bufs=28))
        reppool = ctx.enter_context(tc.tile_pool(name="rep", bufs=28))
        ps_t = ctx.enter_context(tc.tile_pool(name="ps_t", bufs=3, space="PSUM"))
        ps_c = ctx.enter_context(tc.tile_pool(name="ps_c", bufs=1, space="PSUM"))
        ps_r = ctx.enter_context(tc.tile_pool(name="ps_r", bufs=1, space="PSUM"))

        rep = {}     # r-loc key -> [128, n] f16 AP
        uid = [0]
        cpx = {}     # l-loc key -> [16, n] f16 AP (slice of group tile)
        cpgrp = {'ps': None, 'members': [], 'gid': -1}

        def finish_group():
            st = cpgrp
            if st['ps'] is None:
                return
            csb = csbpool.tile([128, 256], F16, tag="csb")
            nc.scalar.copy(csb[:], st['ps'][:, 0:256])
            for (key, q) in st['members']:
                cpx[key] = csb[32 * q:32 * q + 16, :]
            st['ps'] = None
            st['members'] = []

        def emit_pair(li, locs, lgets, rgets, wtile):
            nl = len(locs)
            metas = [LOC_META[(li, y, x)] for (y, x) in locs]
            path = metas[0]['path']
            tp = ps_t.tile([128, 1024], F32, tag="t")
            for i, (y, x) in enumerate(locs):
                a, woff = metas[i]['a'], metas[i]['woff']
                for c in range(2):
                    nc.tensor.matmul(
                        tp[:, i * 512 + c * 256:i * 512 + (c + 1) * 256],
                        wtile[32 * a:32 * a + 16, woff + c * 128:woff + (c + 1) * 128],
                        lgets[i](), start=True, stop=True,
                        tile_position=(32 * a, 0))
            # multiply stage
            msrc = tp
            if path in ('A', 'GA'):
                tsb = tpool.tile([128, 1024], F16, tag="t16")
                nc.scalar.copy(tsb[:, 0:512 * nl], tp[:, 0:512 * nl])
                msrc = tsb
            elif path == 'GD':
                tsb = tpool.tile([128, 1024], F16, tag="t16")
                nc.vector.tensor_copy(tsb[:, 0:512 * nl], tp[:, 0:512 * nl])
                msrc = tsb
            msb = mpool.tile([128, 1024], F16, tag="m16")
            if path in ('GA', 'GD'):
                for i in range(nl):
                    for c in range(2):
                        nc.gpsimd.tensor_mul(
                            msb[:, i * 512 + c * 256:i * 512 + (c + 1) * 256],
                            msrc[:, i * 512 + c * 256:i * 512 + (c + 1) * 256],
                            rgets[i]())
            else:
                for i in range(nl):
                    nc.vector.tensor_mul(
                        msb[:, i * 512:(i + 1) * 512].rearrange("p (c nn) -> p c nn", c=2),
                        msrc[:, i * 512:(i + 1) * 512].rearrange("p (c nn) -> p c nn", c=2),
                        rgets[i]().unsqueeze(1).broadcast_to([128, 2, n]))
            # reduce stage
            for i, (y, x) in enumerate(locs):
                m = metas[i]
                if m['role'] == 'l':
                    st = cpgrp
                    if st['gid'] != m['group']:
                        finish_group()
                        uid[0] += 1
                        st['ps'] = ps_c.tile([128, 512], F32, tag="cp",
                                             name=f"cp{uid[0]}")
                        st['gid'] = m['group']
                    q = m['q']
                    for c in range(2):
                        nc.tensor.matmul(
                            st['ps'][32 * q:32 * q + 16, 0:256],
                            red16[:, 16 * c:16 * c + 16],
                            msb[:, i * 512 + c * 256:i * 512 + (c + 1) * 256],
                            start=(c == 0), stop=(c == 1),
                            tile_position=(0, 32 * q))
                    st['members'].append(((li, y, x), q))
                    if len(st['members']) == GROUPS[m['group']]:
                        finish_group()
                else:
                    uid[0] += 1
                    pr = ps_r.tile([128, 512], F32, tag="r", name=f"pr{uid[0]}")
                    for c in range(2):
                        nc.tensor.matmul(
                            pr[:, 0:256], red[:, c * 128:(c + 1) * 128],
                            msb[:, i * 512 + c * 256:i * 512 + (c + 1) * 256],
                            start=(c == 0), stop=(c == 1))
                    rsb = reppool.tile([128, 256], F16, tag="rep",
                                       name=f"rsb{uid[0]}")
                    nc.scalar.copy(rsb[:], pr[:, 0:256])
                    rep[(li, y, x)] = rsb[:]

        def child_get(key):
            m = LOC_META[key]
            if m['role'] == 'l':
                return lambda k=key: cpx[k]
            return lambda k=key: rep[k]

        for blk in range(NBLK):
            z4c = z4pool.tile([128, 4 * 4 * n], F16, tag="z4e", name="z4ct")
            nc.sync.dma_start(
                z4c[:],
                z4e_h[:].rearrange("p (b f) -> p b f", b=8)[:, blk, :])
            z4rc = []
            for half in range(2):
                zr = z4rpool.tile([128, 8 * 4 * n], F16, tag=f"z4r{half}",
                                  name=f"z4r{half}")
                nc.sync.dma_start(
                    zr[:],
                    z4r_h[:].rearrange("p (b h2 f) -> p b h2 f", b=8, h2=2)
                    [:, blk, half, :])
                z4rc.append(zr)

            def z4_top(r1, c1):
                g, rhi = (((r1 & 7) // 2) + c1) % 4, r1 >> 3
                return (z4c[32 * g:32 * g + 16, :]
                        .rearrange("p (rhi c nn) -> p rhi c nn", rhi=4, c=4)
                        [:, rhi, c1 - 4 * blk, :])

            def z4_bot_rep(y2, c1):
                return (z4rc[y2 // 8][:]
                        .rearrange("p (y c nn) -> p y c nn", y=8, c=4)
                        [:, y2 % 8, c1 - 4 * blk, :])

            wt = {}
            for li in range(2, min(7, maxli + 1)):
                wcols = max(SLABS[(li, blk, a)][1] for a in range(4)
                            if (li, blk, a) in SLABS)
                wt[li] = wpool.tile([128, wcols], F16, tag=f"w{li}", name=f"wt{li}")
                for a in range(4):
                    if (li, blk, a) not in SLABS:
                        continue
                    start, ncol = SLABS[(li, blk, a)]
                    if ncol == 0:
                        continue
                    getattr(nc, wq).dma_start(
                        wt[li][32 * a:32 * a + 16, 0:ncol],
                        wh[li][32 * a:32 * a + 16, start:start + ncol])

            for li, locs in schedule_blk(blk):
                if li > maxli:
                    continue
                lgets, rgets = [], []
                for (y, x) in locs:
                    if li == 2:
                        lgets.append(lambda yy=y, xx=x: z4_top(2 * yy, xx))
                        rgets.append(lambda yy=y, xx=x: z4_bot_rep(yy, xx))
                    else:
                        orient = SPECS[li][4]
                        cl = (li - 1, y, 2 * x) if orient else (li - 1, 2 * y, x)
                        cr = (li - 1, y, 2 * x + 1) if orient else (li - 1, 2 * y + 1, x)
                        lgets.append(child_get(cl))
                        rgets.append(child_get(cr))
                emit_pair(li, locs, lgets, rgets, wt[li])

        for li_w in range(7, 11):
            if li_w > maxli:
                continue
            wcols = max(SLABS[(li_w, 0, a)][1] for a in range(4)
                        if (li_w, 0, a) in SLABS)
            wtg = wpool.tile([128, wcols], F16, tag="wtail", name=f"wtg{li_w}")
            for a in range(4):
                if (li_w, 0, a) not in SLABS:
                    continue
                start, ncol = SLABS[(li_w, 0, a)]
                if ncol == 0:
                    continue
                getattr(nc, wq).dma_start(
                    wtg[32 * a:32 * a + 16, 0:ncol],
                    wh[li_w][32 * a:32 * a + 16, start:start + ncol])
            for li, locs in schedule_tail():
                if li != li_w:
                    continue
                lgets, rgets = [], []
                for (y, x) in locs:
                    orient = SPECS[li][4]
                    cl = (li - 1, y, 2 * x) if orient else (li - 1, 2 * y, x)
                    cr = (li - 1, y, 2 * x + 1) if orient else (li - 1, 2 * y + 1, x)
                    lgets.append(child_get(cl))
                    rgets.append(child_get(cr))
                emit_pair(li, locs, lgets, rgets, wtg)

        if maxli >= 11:
            pt = ps_r.tile([128, 512], F32, tag="r", name="pt11")
            nc.tensor.matmul(pt[0:16, 0:256], w11t[32 * q11:32 * q11 + 16, :],
                             cpx[(10, 0, 0)], start=True, stop=True,
                             tile_position=(32 * q11, 0))
            m11 = mpool.tile([16, 256], F16, tag="m11x", name="m11")
            nc.vector.tensor_mul(m11[:], pt[0:16, 0:256], rep[(10, 0, 1)][0:16, :])
            pf = ps_c.tile([128, 512], F32, tag="cp", name="pf")
            nc.tensor.matmul(pf[0:1, 0:256], ones16[0:16, 0:1], m11[:],
                             start=True, stop=True, tile_position=(0, 0))
            osb = tpool.tile([1, 256], F32, tag="outs")
            nc.scalar.copy(osb[:], pf[0:1, 0:256])
            nc.sync.dma_start(out_h[:], osb[:])
        else:
            zz = tpool.tile([1, 256], F32, tag="outs", name="zz")
            nc.any.memset(zz[:], 0.0)
            nc.sync.dma_start(out_h[:], zz[:])
        if dbg is not None:
            dsb = tpool.tile([128, 256], F16, tag="dbgt", name="dbgt")
            if dbg in rep:
                nc.vector.tensor_copy(dsb[:], rep[dbg])
            else:
                nc.any.memset(dsb[:], 0.0)
                nc.vector.tensor_copy(dsb[0:16, :], cpx[dbg])
            nc.sync.dma_start(dbg_h[:], dsb[:])
    nc.compile()
    return nc


def _get_program():
    global _PROGRAM
    if _PROGRAM is None:
        _PROGRAM = build_program()
    return _PROGRAM


def make_inputs(x, ws, core):
    whbm = pack_weights(ws)
    base = {f"w{li}": whbm[li] for li in range(2, 11)}
    base["w11"] = whbm[11]
    base.update(build_consts())
    z4e, z4r = pack_z4(x[core * NSH:(core + 1) * NSH])
    base["z4e"], base["z4r"] = z4e, z4r
    return base


def kernel(**inputs):
    from concourse.bass_utils import run_bass_kernel_spmd
    x = np.asarray(inputs['x'])
    ws = [np.asarray(inputs[f'w{i}']) for i in range(12)]
    whbm = pack_weights(ws)
    consts = build_consts()
    nc = _get_program()
    base = {f"w{li}": whbm[li] for li in range(2, 11)}
    base["w11"] = whbm[11]
    base.update(consts)
    in_maps = []
    for core in range(NCORES):
        z4e, z4r = pack_z4(x[core * NSH:(core + 1) * NSH])
        m = dict(base)
        m["z4e"], m["z4r"] = z4e, z4r
        in_maps.append(m)
    res = run_bass_kernel_spmd(nc, in_maps, list(range(NCORES)))
    out = np.concatenate([res.results[c]["out"].reshape(NSH)
                          for c in range(NCORES)])
    return out.reshape(BATCH, 1, 1, 1).astype(np.float32)



# revision 4
# speedup vs baseline: 1.0244x; 1.0244x over previous
"""Trainium2 Bass kernel for nn_BinaryTTN (batch 2048, 12-layer binary tree
tensor network), data-parallel across 8 NeuronCores.

Structure (per core, n=256 samples):
  * Layers 0+1 folded on host into layer-2 weights; the 16 z4 monomials per
    2x2 patch are pure input packing and are computed on host (fp32, cast to
    fp16) and DMA'd in, in the layout the layer-2 matmuls consume.
  * Each tree location (li=2..10): t = Wfold^T l (PE, K=16 row-tiled, rhs is
    the left child's compact [16,n] output), then an elementwise multiply by
    the right child's REP-form [128,n] (8x replicated rows), split across
    DVE/ACT/GPSIMD paths, then a PE reduction over j:
      - locations consumed as LEFT by their parent reduce via a col-tiled
        [K=128, M=16] matmul into a shared compact PSUM bank (4 locs/bank),
        evacuated once per 4 locs -> compact [16,n] fp16.
      - locations consumed as RIGHT reduce via the M=128 replicating matmul
        (baseline 'red') -> REP form, evacuated per loc.
  * Multiply paths (per pair of locations, statically assigned):
      D: DVE reads t from PSUM fp32 directly (1x mode)
      A: ACT evacuates t to fp16 SBUF, DVE multiplies at 2x
      GD: DVE copies t PSUM->SBUF, GPSIMD multiplies
      GA: ACT evacuates, GPSIMD multiplies
"""
import sys
import numpy as np

sys.path.insert(0, '/opt/trn_rl_repo')

BATCH, EMBED, H0, W0 = 2048, 2, 64, 64
NCORES = 8
NSH = BATCH // NCORES      # 256
NBLK = 8

# mult-path pattern, cycled over loc-pairs:
#   S   = fused evac+mult on DVE (scalar_tensor_tensor from PSUM, 1x)
#   A   = ACT evac -> DVE dense per-chunk mults (2x)
#   AG  = ACT evac -> DVE chunk0 + GPSIMD chunk1
#   AGG = ACT evac -> GPSIMD both chunks
PATH_PATTERN = ['S', 'A', 'S', 'AG', 'S', 'AGG', 'S', 'A',
                'S', 'AG', 'S', 'AG', 'A', 'S', 'AG', 'S']


def layer_specs():
    out = []
    H, W, ind = H0, W0, EMBED
    for li in range(12):
        bond = 1 if li == 11 else 16
        o = H < W
        h = H // (1 if o else 2)
        w = W // (2 if o else 1)
        out.append((h, w, bond, ind, o))
        H, W, ind = h, w, bond
    return out


SPECS = layer_specs()


def role_of(li, y, x):
    """'l' or 'r': how the parent consumes this loc's output."""
    if li == 11:
        return 'l'
    o_p = SPECS[li + 1][4]
    if o_p:
        return 'l' if x % 2 == 0 else 'r'
    return 'l' if y % 2 == 0 else 'r'


def schedule_blk(blk):
    """Yields (li, [locs...]) pair-batches; four independent y2 rows are
    interleaved so the scheduler always has independent work nearby."""
    for y2 in range(0, 16, 4):
        for c0 in range(0, 4, 2):
            for dy in range(4):
                yield (2, [(y2 + dy, 4 * blk + c0), (y2 + dy, 4 * blk + c0 + 1)])
        for dy in range(4):
            yield (3, [(y2 + dy, 2 * blk), (y2 + dy, 2 * blk + 1)])
        y4 = y2 // 2
        yield (4, [(y4, 2 * blk), (y4, 2 * blk + 1)])
        yield (4, [(y4 + 1, 2 * blk), (y4 + 1, 2 * blk + 1)])
        yield (5, [(y4, blk)])
        yield (5, [(y4 + 1, blk)])
        if y2 == 4:
            yield (6, [(0, blk), (1, blk)])
        elif y2 == 12:
            yield (6, [(2, blk), (3, blk)])


def schedule_tail():
    for li in range(7, 11):
        h, w = SPECS[li][0], SPECS[li][1]
        locs = [(y, x) for y in range(h) for x in range(w)]
        for i0 in range(0, len(locs), 2):
            yield (li, locs[i0:i0 + 2])


def _build_structure():
    """Walks the schedule; assigns per-loc meta:
      role, a (t-MM row group = q of left child / g of z4 slice),
      q+group (l-locs), path (per pair), slab offset per (li, blk, a)."""
    meta = {}
    slab_off = {}
    slabs = {}

    def loc_children(li, y, x):
        orient = SPECS[li][4]
        cl = (li - 1, y, 2 * x) if orient else (li - 1, 2 * y, x)
        cr = (li - 1, y, 2 * x + 1) if orient else (li - 1, 2 * y + 1, x)
        return cl, cr

    # pass 1: emission order + consumer batch index per loc
    batches = []
    for blk in range(NBLK):
        for li, locs in schedule_blk(blk):
            batches.append((li, locs, blk))
        batches.append(None)            # group-flush boundary
    for li, locs in schedule_tail():
        batches.append((li, locs, 0))
    batches.append(None)

    consumed_at = {}
    for bi, b in enumerate(batches):
        if b is None:
            continue
        li, locs, blk = b
        for (y, x) in locs:
            if li >= 3:
                cl, cr = loc_children(li, y, x)
                consumed_at[cl] = bi
                consumed_at[cr] = bi

    # pass 2: consumer-aware group assignment
    state = {'group': 0, 'pair': 0}
    groups = {}
    open_members = []

    def close_group():
        if open_members:
            groups[state['group']] = len(open_members)
            state['group'] += 1
            open_members.clear()

    def emit(li, locs, blk, bi):
        path = PATH_PATTERN[state['pair'] % len(PATH_PATTERN)]
        state['pair'] += 1
        for (y, x) in locs:
            if li == 2:
                a = (y + x) % 4
            else:
                cl, _ = loc_children(li, y, x)
                a = meta[cl]['q']
            off = slab_off.get((li, a), 0)
            slab_off[(li, a)] = off + 256
            if (li, blk, a) not in slabs:
                slabs[(li, blk, a)] = off      # start col for this blk's slab
            m = dict(role=role_of(li, y, x), a=a, blk=blk, path=path,
                     woff=off - slabs[(li, blk, a)])
            if m['role'] == 'l':
                m['q'] = len(open_members)
                m['group'] = state['group']
                open_members.append((li, y, x))
                if len(open_members) == 4:
                    close_group()
            meta[(li, y, x)] = m

    for bi, b in enumerate(batches):
        if b is None:
            close_group()
            continue
        li, locs, blk = b
        emit(li, locs, blk, bi)
        # close if the next batch consumes any open member
        nxt = batches[bi + 1] if bi + 1 < len(batches) else None
        if open_members and nxt is not None:
            if any(consumed_at.get(k, 10 ** 9) <= bi + 1 for k in open_members):
                close_group()

    # sanity: every loc's group must close before its consumer batch
    close_bi = {}
    cnt = {}
    for bi, b in enumerate(batches):
        if b is None:
            continue
        li, locs, blk = b
        for (y, x) in locs:
            m = meta[(li, y, x)]
            if 'group' in m:
                g = m['group']
                cnt[g] = cnt.get(g, 0) + 1
                if cnt[g] == groups[g]:
                    close_bi[g] = bi
    for (li, y, x), m in meta.items():
        if 'group' in m and (li, y, x) in consumed_at:
            assert close_bi[m['group']] < consumed_at[(li, y, x)], \
                f"group {m['group']} closes too late for {(li, y, x)}"

    slabsz = {}
    for (li, blk, a), start in slabs.items():
        end = slab_off[(li, a)]
        nxt = min((s for (li2, b2, a2), s in slabs.items()
                   if li2 == li and a2 == a and s > start), default=end)
        slabsz[(li, blk, a)] = (start, nxt - start)
    shapes = {li: max(slab_off.get((li, a), 0) for a in range(4))
              for li in range(2, 11)}
    return meta, slabsz, shapes, groups


LOC_META, SLABS, WSHAPES, GROUPS = _build_structure()


# ---------------- host packing ----------------
def fold_weights(ws):
    W0m = ws[0].reshape(32, 64, 16, 4)
    T1 = np.einsum('rcbij,rcik,rcjl->rcbkl',
                   ws[1], W0m[:, 0::2], W0m[:, 1::2]).reshape(32, 32, 16, 16)
    Q2 = np.einsum('rcbij,rcik,rcjl->rcbkl', ws[2], T1[0::2], T1[1::2])
    folded = {}
    q = Q2.transpose(0, 1, 3, 2, 4).reshape(16, 32, 16, 2, 8, 16)
    folded[2] = q.transpose(0, 1, 3, 2, 4, 5).reshape(16, 32, 2, 16, 128)
    for li in range(3, 11):
        h, w = SPECS[li][0], SPECS[li][1]
        q = ws[li].transpose(0, 1, 3, 2, 4).reshape(h, w, 16, 2, 8, 16)
        folded[li] = q.transpose(0, 1, 3, 2, 4, 5).reshape(h, w, 2, 16, 128)
    folded[11] = ws[11].transpose(0, 1, 3, 2, 4).reshape(16, 16)
    return folded


def pack_weights(ws):
    folded = fold_weights(ws)
    whbm = {}
    for li in range(2, 11):
        arr = np.zeros((128, WSHAPES[li]), dtype=np.float16)
        for (lli, y, x), m in LOC_META.items():
            if lli != li:
                continue
            a = m['a']
            off = SLABS[(li, m['blk'], a)][0] + m['woff']
            arr[32 * a:32 * a + 16, off:off + 256] = \
                folded[li][y, x].transpose(1, 0, 2).reshape(16, 256)
        whbm[li] = arr
    whbm[11] = folded[11].astype(np.float16)
    return whbm


def build_consts():
    c = {}
    red = np.zeros((128, 256), dtype=np.float16)
    for ch in range(2):
        for bl in range(8):
            b = ch * 8 + bl
            for k in range(16):
                for s in range(8):
                    red[bl * 16 + k, ch * 128 + s * 16 + b] = 1.0
    c['red'] = red
    red16 = np.zeros((128, 32), dtype=np.float16)
    for b in range(16):
        bl, ch = b % 8, b // 8
        for j in range(16):
            red16[bl * 16 + j, ch * 16 + b] = 1.0
    c['red16'] = red16
    repk = np.zeros((128, 128), dtype=np.float16)
    for a in range(4):
        for k in range(16):
            for s in range(8):
                repk[32 * a + k, s * 16 + k] = 1.0
    c['repk'] = repk
    o = np.zeros((128, 4), dtype=np.float16)
    o[0:16, 0] = 1.0
    c['ones16'] = o
    return c


def pack_z4(xsh):
    """xsh [n, 2, 64, 64] fp32 -> z4e/z4o [128, 8*4*4*n], z4r [128, 8*16*4*n].

    z4[n, k=(kl 4, kr 4), r1, c1] = z0L[kl] * z0R[kr], with
    z0[n, (i,j), r, c] = x[n,i,2r,c] * x[n,j,2r+1,c].
    z4e/z4o layout: [32g+k, (blk, rhi, cin, n)] = z4[n, k, rhi*8+2g+par, 4blk+cin]
    z4r layout: [16s+k, (blk, y2, cin, n)] = z4[n, k, 2*y2+1, 4blk+cin]
    """
    n = xsh.shape[0]
    x32 = np.asarray(xsh, dtype=np.float32)
    top = x32[:, :, 0::2, :]
    bot = x32[:, :, 1::2, :]
    z0 = np.einsum('nirc,njrc->nijrc', top, bot).reshape(n, 4, 32, 64)
    z4 = np.einsum('nkrc,nlrc->nklrc', z0[:, :, :, 0::2],
                   z0[:, :, :, 1::2]).reshape(n, 16, 32, 32).astype(np.float16)
    # z4e/z4o: r1 = rhi*8 + g*2 + par
    z4v = z4.reshape(n, 16, 4, 4, 2, 8, 4)  # n, k, rhi, g, par, blk, cin
    arr = z4v[:, :, :, :, 0]  # even rows: n k rhi g blk cin
    arr = arr.transpose(3, 1, 4, 2, 5, 0)  # g k blk rhi cin n
    # rotate partition group by cin so consecutive L2 locs use different
    # PE row groups: slice (y2, c1) lives at group (y2 + c1) % 4
    rot = np.empty_like(arr)
    for cin in range(4):
        rot[:, :, :, :, cin] = np.roll(arr[:, :, :, :, cin], cin, axis=0)
    arr = rot.reshape(4, 16, -1)
    full = np.zeros((4, 32, arr.shape[2]), dtype=np.float16)
    full[:, 0:16] = arr
    z4e = np.ascontiguousarray(full.reshape(128, -1))
    # z4r: bottom rows r1 = 2*y2+1
    zb = z4[:, :, 1::2, :].reshape(n, 16, 16, 8, 4)  # n k y2 blk cin
    zb = zb.transpose(1, 3, 2, 4, 0)  # k blk y2 cin n
    zr = np.broadcast_to(zb[None], (8,) + zb.shape).reshape(128, -1)
    return z4e, np.ascontiguousarray(zr)


# ---------------- device program ----------------
_PROGRAM = None


def build_program(num_devices=NCORES, dbg=None, maxli=11, wq='sync'):
    from contextlib import ExitStack
    import concourse.bass as bass
    import concourse.tile as tile
    from concourse import bacc, mybir

    F16, F32 = mybir.dt.float16, mybir.dt.float32
    n = NSH
    nc = bacc.Bacc("TRN2", target_bir_lowering=False, debug=False,
                   num_devices=num_devices)
    z4e_h = nc.declare_dram_parameter("z4e", [128, 8 * 4 * 4 * n], F16, isOutput=False)
    z4r_h = nc.declare_dram_parameter("z4r", [128, 8 * 16 * 4 * n], F16, isOutput=False)
    wh = {li: nc.declare_dram_parameter(f"w{li}", [128, WSHAPES[li]], F16,
                                        isOutput=False) for li in range(2, 11)}
    wh[11] = nc.declare_dram_parameter("w11", [16, 16], F16, isOutput=False)
    red_h = nc.declare_dram_parameter("red", [128, 256], F16, isOutput=False)
    red16_h = nc.declare_dram_parameter("red16", [128, 32], F16, isOutput=False)
    repk_h = nc.declare_dram_parameter("repk", [128, 128], F16, isOutput=False)
    ones_h = nc.declare_dram_parameter("ones16", [128, 4], F16, isOutput=False)
    out_h = nc.declare_dram_parameter("out", [1, n], F32, isOutput=True)
    dbg_h = (nc.declare_dram_parameter("dbg", [128, n], F16, isOutput=True)
             if dbg is not None else None)

    q11 = LOC_META[(10, 0, 0)]['q'] if (10, 0, 0) in LOC_META else 0

    with tile.TileContext(nc) as tc, ExitStack() as ctx:
        cpool = ctx.enter_context(tc.tile_pool(name="consts", bufs=1))
        red = cpool.tile([128, 256], F16); nc.sync.dma_start(red[:], red_h[:])
        red16 = cpool.tile([128, 32], F16); nc.sync.dma_start(red16[:], red16_h[:])
        repk = cpool.tile([128, 128], F16); nc.sync.dma_start(repk[:], repk_h[:])
        ones16 = cpool.tile([128, 4], F16); nc.sync.dma_start(ones16[:], ones_h[:])
        w11t = cpool.tile([128, 16], F16)
        nc.sync.dma_start(w11t[32 * q11:32 * q11 + 16, :], wh[11][:])

        z4pool = ctx.enter_context(tc.tile_pool(name="z4", bufs=2))
        z4rpool = ctx.enter_context(tc.tile_pool(name="z4r", bufs=1))
        wpool = ctx.enter_context(tc.tile_pool(name="w", bufs=2))
        tpool = ctx.enter_context(tc.tile_pool(name="t", bufs=6))
        mpool = ctx.enter_context(tc.tile_pool(name="m", bufs=6))
        csbpool = ctx.enter_context(tc.tile_pool(name="csb", bufs=28))
        reppool = ctx.enter_context(tc.tile_pool(name="rep", bufs=28))
        ps_t = ctx.enter_context(tc.tile_pool(name="ps_t", bufs=3, space="PSUM"))
        ps_c = ctx.enter_context(tc.tile_pool(name="ps_c", bufs=1, space="PSUM"))
        ps_r = ctx.enter_context(tc.tile_pool(name="ps_r", bufs=1, space="PSUM"))

        # HAM warmup: ~4us of dummy matmuls so the PE clock-gate opens
        # (K=8/8, 2.4 GHz) before the real work starts; they overlap the
        # initial z4/weight DMAs.
        warm_ps = ps_t.tile([128, 1024], F32, tag="t", name="warm")
        for _ in range(36):
            nc.tensor.matmul(warm_ps[:, 0:128], red[:, 0:128], red[:, 0:128],
                             start=True, stop=True)

        rep = {}     # r-loc key -> [128, n] f16 AP
        uid = [0]
        cpx = {}     # l-loc key -> [16, n] f16 AP (slice of group tile)
        cpgrp = {'ps': None, 'members': [], 'gid': -1}

        def finish_group():
            st = cpgrp
            if st['ps'] is None:
                return
            csb = csbpool.tile([128, 256], F16, tag="csb")
            nc.scalar.copy(csb[:], st['ps'][:, 0:256])
            for (key, q) in st['members']:
                cpx[key] = csb[32 * q:32 * q + 16, :]
            st['ps'] = None
            st['members'] = []

        def emit_pair(li, locs, lgets, rgets, wtile):
            nl = len(locs)
            metas = [LOC_META[(li, y, x)] for (y, x) in locs]
            path = metas[0]['path']
            tp = ps_t.tile([128, 1024], F32, tag="t")
            for i, (y, x) in enumerate(locs):
                a, woff = metas[i]['a'], metas[i]['woff']
                for c in range(2):
                    nc.tensor.matmul(
                        tp[:, i * 512 + c * 256:i * 512 + (c + 1) * 256],
                        wtile[32 * a:32 * a + 16, woff + c * 128:woff + (c + 1) * 128],
                        lgets[i](), start=True, stop=True,
                        tile_position=(32 * a, 0))
            # multiply stage
            msb = mpool.tile([128, 1024], F16, tag="m16")
            if path == 'S':
                # fused PSUM-read + multiply on DVE; no ACT evac needed
                for i in range(nl):
                    nc.vector.scalar_tensor_tensor(
                        msb[:, i * 512:(i + 1) * 512].rearrange("p (c nn) -> p c nn", c=2),
                        tp[:, i * 512:(i + 1) * 512].rearrange("p (c nn) -> p c nn", c=2),
                        1.0,
                        rgets[i]().unsqueeze(1).broadcast_to([128, 2, n]),
                        op0=mybir.AluOpType.mult, op1=mybir.AluOpType.mult)
            else:
                tsb = tpool.tile([128, 1024], F16, tag="t16")
                nc.scalar.copy(tsb[:, 0:512 * nl], tp[:, 0:512 * nl])
                for i in range(nl):
                    r = rgets[i]()
                    for c in range(2):
                        dst = msb[:, i * 512 + c * 256:i * 512 + (c + 1) * 256]
                        src = tsb[:, i * 512 + c * 256:i * 512 + (c + 1) * 256]
                        on_dve = path == 'A' or (path == 'AG' and c == 0)
                        (nc.vector if on_dve else nc.gpsimd).tensor_mul(dst, src, r)
            # reduce stage
            for i, (y, x) in enumerate(locs):
                m = metas[i]
                if m['role'] == 'l':
                    st = cpgrp
                    if st['gid'] != m['group']:
                        finish_group()
                        uid[0] += 1
                        st['ps'] = ps_c.tile([128, 512], F32, tag="cp",
                                             name=f"cp{uid[0]}")
                        st['gid'] = m['group']
                    q = m['q']
                    for c in range(2):
                        nc.tensor.matmul(
                            st['ps'][32 * q:32 * q + 16, 0:256],
                            red16[:, 16 * c:16 * c + 16],
                            msb[:, i * 512 + c * 256:i * 512 + (c + 1) * 256],
                            start=(c == 0), stop=(c == 1),
                            tile_position=(0, 32 * q))
                    st['members'].append(((li, y, x), q))
                    if len(st['members']) == GROUPS[m['group']]:
                        finish_group()
                else:
                    uid[0] += 1
                    pr = ps_r.tile([128, 512], F32, tag="r", name=f"pr{uid[0]}")
                    for c in range(2):
                        nc.tensor.matmul(
                            pr[:, 0:256], red[:, c * 128:(c + 1) * 128],
                            msb[:, i * 512 + c * 256:i * 512 + (c + 1) * 256],
                            start=(c == 0), stop=(c == 1))
                    rsb = reppool.tile([128, 256], F16, tag="rep",
                                       name=f"rsb{uid[0]}")
                    nc.scalar.copy(rsb[:], pr[:, 0:256])
                    rep[(li, y, x)] = rsb[:]

        def child_get(key):
            m = LOC_META[key]
            if m['role'] == 'l':
                return lambda k=key: cpx[k]
            return lambda k=key: rep[k]

        for blk in range(NBLK):
            z4c = z4pool.tile([128, 4 * 4 * n], F16, tag="z4e", name="z4ct")
            nc.sync.dma_start(
                z4c[:],
                z4e_h[:].rearrange("p (b f) -> p b f", b=8)[:, blk, :])
            z4rc = []
            for half in range(2):
                zr = z4rpool.tile([128, 8 * 4 * n], F16, tag=f"z4r{half}",
                                  name=f"z4r{half}")
                nc.sync.dma_start(
                    zr[:],
                    z4r_h[:].rearrange("p (b h2 f) -> p b h2 f", b=8, h2=2)
                    [:, blk, half, :])
                z4rc.append(zr)

            def z4_top(r1, c1):
                g, rhi = (((r1 & 7) // 2) + c1) % 4, r1 >> 3
                return (z4c[32 * g:32 * g + 16, :]
                        .rearrange("p (rhi c nn) -> p rhi c nn", rhi=4, c=4)
                        [:, rhi, c1 - 4 * blk, :])

            def z4_bot_rep(y2, c1):
                return (z4rc[y2 // 8][:]
                        .rearrange("p (y c nn) -> p y c nn", y=8, c=4)
                        [:, y2 % 8, c1 - 4 * blk, :])

            wt = {}
            for li in range(2, min(7, maxli + 1)):
                wcols = max(SLABS[(li, blk, a)][1] for a in range(4)
                            if (li, blk, a) in SLABS)
                wt[li] = wpool.tile([128, wcols], F16, tag=f"w{li}", name=f"wt{li}")
                for a in range(4):
                    if (li, blk, a) not in SLABS:
                        continue
                    start, ncol = SLABS[(li, blk, a)]
                    if ncol == 0:
                        continue
                    getattr(nc, wq).dma_start(
                        wt[li][32 * a:32 * a + 16, 0:ncol],
                        wh[li][32 * a:32 * a + 16, start:start + ncol])

            for li, locs in schedule_blk(blk):
                if li > maxli:
                    continue
                lgets, rgets = [], []
                for (y, x) in locs:
                    if li == 2:
                        lgets.append(lambda yy=y, xx=x: z4_top(2 * yy, xx))
                        rgets.append(lambda yy=y, xx=x: z4_bot_rep(yy, xx))
                    else:
                        orient = SPECS[li][4]
                        cl = (li - 1, y, 2 * x) if orient else (li - 1, 2 * y, x)
                        cr = (li - 1, y, 2 * x + 1) if orient else (li - 1, 2 * y + 1, x)
                        lgets.append(child_get(cl))
                        rgets.append(child_get(cr))
                emit_pair(li, locs, lgets, rgets, wt[li])

        for li_w in range(7, 11):
            if li_w > maxli:
                continue
            wcols = max(SLABS[(li_w, 0, a)][1] for a in range(4)
                        if (li_w, 0, a) in SLABS)
            wtg = wpool.tile([128, wcols], F16, tag="wtail", name=f"wtg{li_w}")
            for a in range(4):
                if (li_w, 0, a) not in SLABS:
                    continue
                start, ncol = SLABS[(li_w, 0, a)]
                if ncol == 0:
                    continue
                getattr(nc, wq).dma_start(
                    wtg[32 * a:32 * a + 16, 0:ncol],
                    wh[li_w][32 * a:32 * a + 16, start:start + ncol])
            for li, locs in schedule_tail():
                if li != li_w:
                    continue
                lgets, rgets = [], []
                for (y, x) in locs:
                    orient = SPECS[li][4]
                    cl = (li - 1, y, 2 * x) if orient else (li - 1, 2 * y, x)
                    cr = (li - 1, y, 2 * x + 1) if orient else (li - 1, 2 * y + 1, x)
                    lgets.append(child_get(cl))
                    rgets.append(child_get(cr))
                emit_pair(li, locs, lgets, rgets, wtg)

        if maxli >= 11:
            pt = ps_r.tile([128, 512], F32, tag="r", name="pt11")
            nc.tensor.matmul(pt[0:16, 0:256], w11t[32 * q11:32 * q11 + 16, :],
                             cpx[(10, 0, 0)], start=True, stop=True,
                             tile_position=(32 * q11, 0))
            m11 = mpool.tile([16, 256], F16, tag="m11x", name="m11")
            nc.vector.tensor_mul(m11[:], pt[0:16, 0:256], rep[(10, 0, 1)][0:16, :])
            pf = ps_c.tile([128, 512], F32, tag="cp", name="pf")
            nc.tensor.matmul(pf[0:1, 0:256], ones16[0:16, 0:1], m11[:],
                             start=True, stop=True, tile_position=(0, 0))
            osb = tpool.tile([1, 256], F32, tag="outs")
            nc.scalar.copy(osb[:], pf[0:1, 0:256])
            nc.sync.dma_start(out_h[:], osb[:])
        else:
            zz = tpool.tile([1, 256], F32, tag="outs", name="zz")
            nc.any.memset(zz[:], 0.0)
            nc.sync.dma_start(out_h[:], zz[:])
        if dbg is not None:
            dsb = tpool.tile([128, 256], F16, tag="dbgt", name="dbgt")
            if dbg in rep:
                nc.vector.tensor_copy(dsb[:], rep[dbg])
            else:
                nc.any.memset(dsb[:], 0.0)
                nc.vector.tensor_copy(dsb[0:16, :], cpx[dbg])
            nc.sync.dma_start(dbg_h[:], dsb[:])
    nc.compile()
    return nc


def _get_program():
    global _PROGRAM
    if _PROGRAM is None:
        _PROGRAM = build_program()
    return _PROGRAM


def make_inputs(x, ws, core):
    whbm = pack_weights(ws)
    base = {f"w{li}": whbm[li] for li in range(2, 11)}
    base["w11"] = whbm[11]
    base.update(build_consts())
    z4e, z4r = pack_z4(x[core * NSH:(core + 1) * NSH])
    base["z4e"], base["z4r"] = z4e, z4r
    return base


def kernel(**inputs):
    from concourse.bass_utils import run_bass_kernel_spmd
    x = np.asarray(inputs['x'])
    ws = [np.asarray(inputs[f'w{i}']) for i in range(12)]
    whbm = pack_weights(ws)
    consts = build_consts()
    nc = _get_program()
    base = {f"w{li}": whbm[li] for li in range(2, 11)}
    base["w11"] = whbm[11]
    base.update(consts)
    in_maps = []
    for core in range(NCORES):
        z4e, z4r = pack_z4(x[core * NSH:(core + 1) * NSH])
        m = dict(base)
        m["z4e"], m["z4r"] = z4e, z4r
        in_maps.append(m)
    res = run_bass_kernel_spmd(nc, in_maps, list(range(NCORES)))
    out = np.concatenate([res.results[c]["out"].reshape(NSH)
                          for c in range(NCORES)])
    return out.reshape(BATCH, 1, 1, 1).astype(np.float32)



# revision 13
# speedup vs baseline: 1.0716x; 1.0460x over previous
"""Trainium2 Bass kernel for nn_BinaryTTN (batch 2048, 12-layer binary tree
tensor network), data-parallel across 8 NeuronCores.

Structure (per core, n=256 samples):
  * Layers 0+1 folded on host into layer-2 weights; the 16 z4 monomials per
    2x2 patch are pure input packing and are computed on host (fp32, cast to
    fp16) and DMA'd in, in the layout the layer-2 matmuls consume.
  * Each tree location (li=2..10): t = Wfold^T l (PE, K=16 row-tiled, rhs is
    the left child's compact [16,n] output), then an elementwise multiply by
    the right child's REP-form [128,n] (8x replicated rows), split across
    DVE/ACT/GPSIMD paths, then a PE reduction over j:
      - locations consumed as LEFT by their parent reduce via a col-tiled
        [K=128, M=16] matmul into a shared compact PSUM bank (4 locs/bank),
        evacuated once per 4 locs -> compact [16,n] fp16.
      - locations consumed as RIGHT reduce via the M=128 replicating matmul
        (baseline 'red') -> REP form, evacuated per loc.
  * Multiply paths (per pair of locations, statically assigned):
      D: DVE reads t from PSUM fp32 directly (1x mode)
      A: ACT evacuates t to fp16 SBUF, DVE multiplies at 2x
      GD: DVE copies t PSUM->SBUF, GPSIMD multiplies
      GA: ACT evacuates, GPSIMD multiplies
"""
import sys
import numpy as np
import ml_dtypes

BF16 = ml_dtypes.bfloat16

sys.path.insert(0, '/opt/trn_rl_repo')

BATCH, EMBED, H0, W0 = 2048, 2, 64, 64
NCORES = 8
NSH = BATCH // NCORES      # 256
NBLK = 8

# mult-path pattern, cycled over loc-pairs:
#   S   = fused evac+mult on DVE (scalar_tensor_tensor from PSUM, 1x)
#   A   = ACT evac -> DVE dense per-chunk mults (2x)
#   AG  = ACT evac -> DVE chunk0 + GPSIMD chunk1
#   AGG = ACT evac -> GPSIMD both chunks
PATH_PATTERN = ['S', 'A', 'S', 'AG', 'S', 'AGG', 'S', 'A',
                'S', 'AG', 'S', 'AG', 'A', 'S', 'AG', 'S']


def layer_specs():
    out = []
    H, W, ind = H0, W0, EMBED
    for li in range(12):
        bond = 1 if li == 11 else 16
        o = H < W
        h = H // (1 if o else 2)
        w = W // (2 if o else 1)
        out.append((h, w, bond, ind, o))
        H, W, ind = h, w, bond
    return out


SPECS = layer_specs()


def role_of(li, y, x):
    """'l' or 'r': how the parent consumes this loc's output."""
    if li == 11:
        return 'l'
    o_p = SPECS[li + 1][4]
    if o_p:
        return 'l' if x % 2 == 0 else 'r'
    return 'l' if y % 2 == 0 else 'r'


def schedule_blk(blk):
    """Yields (li, [locs...]) pair-batches; four independent y2 rows are
    interleaved so the scheduler always has independent work nearby."""
    for y2 in range(0, 16, 4):
        for c0 in range(0, 4, 2):
            for dy in range(4):
                yield (2, [(y2 + dy, 4 * blk + c0), (y2 + dy, 4 * blk + c0 + 1)])
        for dy in range(4):
            yield (3, [(y2 + dy, 2 * blk), (y2 + dy, 2 * blk + 1)])
        y4 = y2 // 2
        yield (4, [(y4, 2 * blk), (y4, 2 * blk + 1)])
        yield (4, [(y4 + 1, 2 * blk), (y4 + 1, 2 * blk + 1)])
        yield (5, [(y4, blk)])
        yield (5, [(y4 + 1, blk)])
        if y2 == 4:
            yield (6, [(0, blk), (1, blk)])
        elif y2 == 12:
            yield (6, [(2, blk), (3, blk)])


def schedule_tail():
    for li in range(7, 11):
        h, w = SPECS[li][0], SPECS[li][1]
        locs = [(y, x) for y in range(h) for x in range(w)]
        for i0 in range(0, len(locs), 2):
            yield (li, locs[i0:i0 + 2])


def _build_structure():
    """Walks the schedule; assigns per-loc meta:
      role, a (t-MM row group = q of left child / g of z4 slice),
      q+group (l-locs), path (per pair), slab offset per (li, blk, a)."""
    meta = {}
    slab_off = {}
    slabs = {}

    def loc_children(li, y, x):
        orient = SPECS[li][4]
        cl = (li - 1, y, 2 * x) if orient else (li - 1, 2 * y, x)
        cr = (li - 1, y, 2 * x + 1) if orient else (li - 1, 2 * y + 1, x)
        return cl, cr

    # pass 1: emission order + consumer batch index per loc
    batches = []
    for blk in range(NBLK):
        for li, locs in schedule_blk(blk):
            batches.append((li, locs, blk))
        batches.append(None)            # group-flush boundary
    for li, locs in schedule_tail():
        batches.append((li, locs, 0))
    batches.append(None)

    consumed_at = {}
    for bi, b in enumerate(batches):
        if b is None:
            continue
        li, locs, blk = b
        for (y, x) in locs:
            if li >= 3:
                cl, cr = loc_children(li, y, x)
                consumed_at[cl] = bi
                consumed_at[cr] = bi

    # pass 2: consumer-aware group assignment
    state = {'group': 0, 'pair': 0}
    groups = {}
    open_members = []

    def close_group():
        if open_members:
            groups[state['group']] = len(open_members)
            state['group'] += 1
            open_members.clear()

    def emit(li, locs, blk, bi):
        path = PATH_PATTERN[state['pair'] % len(PATH_PATTERN)]
        state['pair'] += 1
        for (y, x) in locs:
            if li == 2:
                a = (y + x) % 4
            else:
                cl, _ = loc_children(li, y, x)
                a = meta[cl]['q']
            off = slab_off.get((li, a), 0)
            slab_off[(li, a)] = off + 256
            if (li, blk, a) not in slabs:
                slabs[(li, blk, a)] = off      # start col for this blk's slab
            m = dict(role=role_of(li, y, x), a=a, blk=blk, path=path,
                     woff=off - slabs[(li, blk, a)])
            if m['role'] == 'l':
                m['q'] = len(open_members)
                m['group'] = state['group']
                open_members.append((li, y, x))
                if len(open_members) == 4:
                    close_group()
            meta[(li, y, x)] = m

    for bi, b in enumerate(batches):
        if b is None:
            close_group()
            continue
        li, locs, blk = b
        emit(li, locs, blk, bi)
        # close if the next batch consumes any open member
        nxt = batches[bi + 1] if bi + 1 < len(batches) else None
        if open_members and nxt is not None:
            if any(consumed_at.get(k, 10 ** 9) <= bi + 1 for k in open_members):
                close_group()

    # sanity: every loc's group must close before its consumer batch
    close_bi = {}
    cnt = {}
    for bi, b in enumerate(batches):
        if b is None:
            continue
        li, locs, blk = b
        for (y, x) in locs:
            m = meta[(li, y, x)]
            if 'group' in m:
                g = m['group']
                cnt[g] = cnt.get(g, 0) + 1
                if cnt[g] == groups[g]:
                    close_bi[g] = bi
    for (li, y, x), m in meta.items():
        if 'group' in m and (li, y, x) in consumed_at:
            assert close_bi[m['group']] < consumed_at[(li, y, x)], \
                f"group {m['group']} closes too late for {(li, y, x)}"

    slabsz = {}
    for (li, blk, a), start in slabs.items():
        end = slab_off[(li, a)]
        nxt = min((s for (li2, b2, a2), s in slabs.items()
                   if li2 == li and a2 == a and s > start), default=end)
        slabsz[(li, blk, a)] = (start, nxt - start)
    shapes = {li: max(slab_off.get((li, a), 0) for a in range(4))
              for li in range(2, 11)}
    return meta, slabsz, shapes, groups


LOC_META, SLABS, WSHAPES, GROUPS = _build_structure()


# ---------------- host packing ----------------
def fold_weights(ws):
    W0m = ws[0].reshape(32, 64, 16, 4)
    T1 = np.einsum('rcbij,rcik,rcjl->rcbkl',
                   ws[1], W0m[:, 0::2], W0m[:, 1::2]).reshape(32, 32, 16, 16)
    Q2 = np.einsum('rcbij,rcik,rcjl->rcbkl', ws[2], T1[0::2], T1[1::2])
    folded = {}
    q = Q2.transpose(0, 1, 3, 2, 4).reshape(16, 32, 16, 2, 8, 16)
    folded[2] = q.transpose(0, 1, 3, 2, 4, 5).reshape(16, 32, 2, 16, 128)
    for li in range(3, 11):
        h, w = SPECS[li][0], SPECS[li][1]
        q = ws[li].transpose(0, 1, 3, 2, 4).reshape(h, w, 16, 2, 8, 16)
        folded[li] = q.transpose(0, 1, 3, 2, 4, 5).reshape(h, w, 2, 16, 128)
    folded[11] = ws[11].transpose(0, 1, 3, 2, 4).reshape(16, 16)
    return folded


def pack_weights(ws):
    folded = fold_weights(ws)
    whbm = {}
    for li in range(2, 11):
        arr = np.zeros((128, WSHAPES[li]), dtype=BF16)
        for (lli, y, x), m in LOC_META.items():
            if lli != li:
                continue
            a = m['a']
            off = SLABS[(li, m['blk'], a)][0] + m['woff']
            arr[32 * a:32 * a + 16, off:off + 256] = \
                folded[li][y, x].transpose(1, 0, 2).reshape(16, 256)
        whbm[li] = arr
    whbm[11] = folded[11].astype(BF16)
    return whbm


def build_consts():
    c = {}
    red = np.zeros((128, 256), dtype=BF16)
    for ch in range(2):
        for bl in range(8):
            b = ch * 8 + bl
            for k in range(16):
                for s in range(8):
                    red[bl * 16 + k, ch * 128 + s * 16 + b] = 1.0
    c['red'] = red
    red16 = np.zeros((128, 32), dtype=BF16)
    for b in range(16):
        bl, ch = b % 8, b // 8
        for j in range(16):
            red16[bl * 16 + j, ch * 16 + b] = 1.0
    c['red16'] = red16
    repk = np.zeros((128, 128), dtype=BF16)
    for a in range(4):
        for k in range(16):
            for s in range(8):
                repk[32 * a + k, s * 16 + k] = 1.0
    c['repk'] = repk
    o = np.zeros((128, 4), dtype=BF16)
    o[0:16, 0] = 1.0
    c['ones16'] = o
    return c


def pack_z4(xsh):
    """xsh [n, 2, 64, 64] fp32 -> z4e/z4o [128, 8*4*4*n], z4r [128, 8*16*4*n].

    z4[n, k=(kl 4, kr 4), r1, c1] = z0L[kl] * z0R[kr], with
    z0[n, (i,j), r, c] = x[n,i,2r,c] * x[n,j,2r+1,c].
    z4e/z4o layout: [32g+k, (blk, rhi, cin, n)] = z4[n, k, rhi*8+2g+par, 4blk+cin]
    z4r layout: [16s+k, (blk, y2, cin, n)] = z4[n, k, 2*y2+1, 4blk+cin]
    """
    n = xsh.shape[0]
    x32 = np.asarray(xsh, dtype=np.float32)
    top = x32[:, :, 0::2, :]
    bot = x32[:, :, 1::2, :]
    z0 = np.einsum('nirc,njrc->nijrc', top, bot).reshape(n, 4, 32, 64)
    z4 = np.einsum('nkrc,nlrc->nklrc', z0[:, :, :, 0::2],
                   z0[:, :, :, 1::2]).reshape(n, 16, 32, 32).astype(BF16)
    # z4e/z4o: r1 = rhi*8 + g*2 + par
    z4v = z4.reshape(n, 16, 4, 4, 2, 8, 4)  # n, k, rhi, g, par, blk, cin
    arr = z4v[:, :, :, :, 0]  # even rows: n k rhi g blk cin
    arr = arr.transpose(3, 1, 4, 2, 5, 0)  # g k blk rhi cin n
    # rotate partition group by cin so consecutive L2 locs use different
    # PE row groups: slice (y2, c1) lives at group (y2 + c1) % 4
    rot = np.empty_like(arr)
    for cin in range(4):
        rot[:, :, :, :, cin] = np.roll(arr[:, :, :, :, cin], cin, axis=0)
    arr = rot.reshape(4, 16, -1)
    full = np.zeros((4, 32, arr.shape[2]), dtype=BF16)
    full[:, 0:16] = arr
    z4e = np.ascontiguousarray(full.reshape(128, -1))
    # z4r: bottom rows r1 = 2*y2+1
    zb = z4[:, :, 1::2, :].reshape(n, 16, 16, 8, 4)  # n k y2 blk cin
    zb = zb.transpose(1, 3, 2, 4, 0)  # k blk y2 cin n
    zr = np.broadcast_to(zb[None], (8,) + zb.shape).reshape(128, -1)
    return z4e, np.ascontiguousarray(zr)


# ---------------- device program ----------------
_PROGRAM = None


def build_program(num_devices=NCORES, dbg=None, maxli=11, wq='sync'):
    from contextlib import ExitStack
    import concourse.bass as bass
    import concourse.tile as tile
    from concourse import bacc, mybir

    F16, F32 = mybir.dt.bfloat16, mybir.dt.float32
    n = NSH
    nc = bacc.Bacc("TRN2", target_bir_lowering=False, debug=False,
                   num_devices=num_devices)
    z4e_h = nc.declare_dram_parameter("z4e", [128, 8 * 4 * 4 * n], F16, isOutput=False)
    z4r_h = nc.declare_dram_parameter("z4r", [128, 8 * 16 * 4 * n], F16, isOutput=False)
    wh = {li: nc.declare_dram_parameter(f"w{li}", [128, WSHAPES[li]], F16,
                                        isOutput=False) for li in range(2, 11)}
    wh[11] = nc.declare_dram_parameter("w11", [16, 16], F16, isOutput=False)
    red_h = nc.declare_dram_parameter("red", [128, 256], F16, isOutput=False)
    red16_h = nc.declare_dram_parameter("red16", [128, 32], F16, isOutput=False)
    repk_h = nc.declare_dram_parameter("repk", [128, 128], F16, isOutput=False)
    ones_h = nc.declare_dram_parameter("ones16", [128, 4], F16, isOutput=False)
    out_h = nc.declare_dram_parameter("out", [1, n], F32, isOutput=True)
    dbg_h = (nc.declare_dram_parameter("dbg", [128, n], F16, isOutput=True)
             if dbg is not None else None)

    q11 = LOC_META[(10, 0, 0)]['q'] if (10, 0, 0) in LOC_META else 0

    with tile.TileContext(nc) as tc, ExitStack() as ctx:
        cpool = ctx.enter_context(tc.tile_pool(name="consts", bufs=1))
        red = cpool.tile([128, 256], F16); nc.sync.dma_start(red[:], red_h[:])
        red16 = cpool.tile([128, 32], F16); nc.sync.dma_start(red16[:], red16_h[:])
        repk = cpool.tile([128, 128], F16); nc.sync.dma_start(repk[:], repk_h[:])
        ones16 = cpool.tile([128, 4], F16); nc.sync.dma_start(ones16[:], ones_h[:])
        w11t = cpool.tile([128, 16], F16)
        nc.sync.dma_start(w11t[32 * q11:32 * q11 + 16, :], wh[11][:])

        z4pool = ctx.enter_context(tc.tile_pool(name="z4", bufs=2))
        z4rpool = ctx.enter_context(tc.tile_pool(name="z4r", bufs=1))
        wpool = ctx.enter_context(tc.tile_pool(name="w", bufs=2))
        tpool = ctx.enter_context(tc.tile_pool(name="t", bufs=6))
        mpool = ctx.enter_context(tc.tile_pool(name="m", bufs=6))
        csbpool = ctx.enter_context(tc.tile_pool(name="csb", bufs=28))
        reppool = ctx.enter_context(tc.tile_pool(name="rep", bufs=28))
        ps_t = ctx.enter_context(tc.tile_pool(name="ps_t", bufs=5, space="PSUM"))
        ps_c = ctx.enter_context(tc.tile_pool(name="ps_c", bufs=1, space="PSUM"))
        ps_r = ctx.enter_context(tc.tile_pool(name="ps_r", bufs=2, space="PSUM"))

        # HAM warmup: ~4us of dummy matmuls so the PE clock-gate opens
        # (K=8/8, 2.4 GHz) before the real work starts; they overlap the
        # initial z4/weight DMAs.
        warm_ps = ps_t.tile([128, 512], F32, tag="t", name="warm")
        for _ in range(36):
            nc.tensor.matmul(warm_ps[:, 0:128], red[:, 0:128], red[:, 0:128],
                             start=True, stop=True)

        rep = {}     # r-loc key -> [128, n] f16 AP
        uid = [0]
        cpx = {}     # l-loc key -> [16, n] f16 AP (slice of group tile)
        cpgrp = {'ps': None, 'members': [], 'gid': -1}

        def finish_group():
            st = cpgrp
            if st['ps'] is None:
                return
            csb = csbpool.tile([128, 256], F16, tag="csb")
            nc.scalar.copy(csb[:], st['ps'][:, 0:256])
            for (key, q) in st['members']:
                cpx[key] = csb[32 * q:32 * q + 16, :]
            st['ps'] = None
            st['members'] = []

        # ---- wave machinery: batch 2 pairs, emit PE work phase-major so
        # LDWEIGHTS overlaps in-flight matmuls (row/col-group rotation) ----
        wave = []        # entries: dict(li, locs, metas, lgets, rgets, wtile)
        wave_keys = set()

        def flush_wave():
            if not wave:
                return
            # phase 1: t-matmuls, chunk-major across locs -> row groups rotate
            for e in wave:
                tps = []
                for i in range(len(e['locs'])):
                    uid[0] += 1
                    tps.append(ps_t.tile([128, 512], F32, tag="t",
                                         name=f"tp{uid[0]}"))
                e['tps'] = tps
            for c in range(2):
                for e in wave:
                    for i in range(len(e['locs'])):
                        m = e['metas'][i]
                        a, woff = m['a'], m['woff']
                        nc.tensor.matmul(
                            e['tps'][i][:, c * 256:(c + 1) * 256],
                            e['wtile'][32 * a:32 * a + 16,
                                       woff + c * 128:woff + (c + 1) * 128],
                            e['lgets'][i](), start=True, stop=True,
                            tile_position=(32 * a, 0))
            # phase 2: multiplies
            for e in wave:
                nl = len(e['locs'])
                path = e['metas'][0]['path']
                msb = mpool.tile([128, 1024], F16, tag="m16")
                e['msb'] = msb
                if path == 'S':
                    for i in range(nl):
                        nc.vector.scalar_tensor_tensor(
                            msb[:, i * 512:(i + 1) * 512]
                            .rearrange("p (c nn) -> p c nn", c=2),
                            e['tps'][i][:].rearrange("p (c nn) -> p c nn", c=2),
                            1.0,
                            e['rgets'][i]().unsqueeze(1).broadcast_to([128, 2, n]),
                            op0=mybir.AluOpType.mult, op1=mybir.AluOpType.mult)
                else:
                    tsb = tpool.tile([128, 1024], F16, tag="t16")
                    for i in range(nl):
                        nc.scalar.copy(tsb[:, i * 512:(i + 1) * 512],
                                       e['tps'][i][:])
                    for i in range(nl):
                        r = e['rgets'][i]()
                        for c in range(2):
                            dst = msb[:, i * 512 + c * 256:i * 512 + (c + 1) * 256]
                            src = tsb[:, i * 512 + c * 256:i * 512 + (c + 1) * 256]
                            on_dve = path == 'A' or (path == 'AG' and c == 0)
                            (nc.vector if on_dve else nc.gpsimd).tensor_mul(
                                dst, src, r)
            # phase 3: reduces; l-jobs grouped by compact-group (col-rotated),
            # then r-jobs in sub-batches of 2 (ps_r depth)
            ljobs, rjobs = [], []
            for e in wave:
                for i, (y, x) in enumerate(e['locs']):
                    m = e['metas'][i]
                    (ljobs if m['role'] == 'l' else rjobs).append(
                        (e, i, (e['li'], y, x), m))
            # l: contiguous segments share a group id
            k0 = 0
            while k0 < len(ljobs):
                gid = ljobs[k0][3]['group']
                k1 = k0
                while k1 < len(ljobs) and ljobs[k1][3]['group'] == gid:
                    k1 += 1
                seg = ljobs[k0:k1]
                st = cpgrp
                if st['gid'] != gid:
                    finish_group()
                    uid[0] += 1
                    st['ps'] = ps_c.tile([128, 512], F32, tag="cp",
                                         name=f"cp{uid[0]}")
                    st['gid'] = gid
                for c in range(2):
                    for (e, i, key, m) in seg:
                        q = m['q']
                        nc.tensor.matmul(
                            st['ps'][32 * q:32 * q + 16, 0:256],
                            red16[:, 16 * c:16 * c + 16],
                            e['msb'][:, i * 512 + c * 256:i * 512 + (c + 1) * 256],
                            start=(c == 0), stop=(c == 1),
                            tile_position=(0, 32 * q))
                for (e, i, key, m) in seg:
                    st['members'].append((key, m['q']))
                if len(st['members']) == GROUPS[gid]:
                    finish_group()
                k0 = k1
            # r: sub-batches of 2
            for k0 in range(0, len(rjobs), 2):
                sub = rjobs[k0:k0 + 2]
                prs = []
                for _ in sub:
                    uid[0] += 1
                    prs.append(ps_r.tile([128, 512], F32, tag="r",
                                         name=f"pr{uid[0]}"))
                for c in range(2):
                    for (e, i, key, m), pr in zip(sub, prs):
                        nc.tensor.matmul(
                            pr[:, 0:256], red[:, c * 128:(c + 1) * 128],
                            e['msb'][:, i * 512 + c * 256:i * 512 + (c + 1) * 256],
                            start=(c == 0), stop=(c == 1))
                for (e, i, key, m), pr in zip(sub, prs):
                    uid[0] += 1
                    rsb = reppool.tile([128, 256], F16, tag="rep",
                                       name=f"rsb{uid[0]}")
                    nc.scalar.copy(rsb[:], pr[:, 0:256])
                    rep[key] = rsb[:]
            wave.clear()
            wave_keys.clear()

        def emit_pair(li, locs, lgets, rgets, wtile):
            orient = SPECS[li][4]
            if li >= 3:
                for (y, x) in locs:
                    cl = (li - 1, y, 2 * x) if orient else (li - 1, 2 * y, x)
                    cr = (li - 1, y, 2 * x + 1) if orient else (li - 1, 2 * y + 1, x)
                    if cl not in cpx or cr not in rep:
                        flush_wave()
                        break
            wave.append(dict(li=li, locs=locs,
                             metas=[LOC_META[(li, y, x)] for (y, x) in locs],
                             lgets=lgets, rgets=rgets, wtile=wtile))
            for (y, x) in locs:
                wave_keys.add((li, y, x))
            if len(wave) >= 2:
                flush_wave()

        def child_get(key):
            m = LOC_META[key]
            if m['role'] == 'l':
                return lambda k=key: cpx[k]
            return lambda k=key: rep[k]

        for blk in range(NBLK):
            z4c = z4pool.tile([128, 4 * 4 * n], F16, tag="z4e", name="z4ct")
            nc.sync.dma_start(
                z4c[:],
                z4e_h[:].rearrange("p (b f) -> p b f", b=8)[:, blk, :])
            z4rc = []
            for half in range(2):
                zr = z4rpool.tile([128, 8 * 4 * n], F16, tag=f"z4r{half}",
                                  name=f"z4r{half}")
                nc.sync.dma_start(
                    zr[:],
                    z4r_h[:].rearrange("p (b h2 f) -> p b h2 f", b=8, h2=2)
                    [:, blk, half, :])
                z4rc.append(zr)

            def z4_top(r1, c1):
                g, rhi = (((r1 & 7) // 2) + c1) % 4, r1 >> 3
                return (z4c[32 * g:32 * g + 16, :]
                        .rearrange("p (rhi c nn) -> p rhi c nn", rhi=4, c=4)
                        [:, rhi, c1 - 4 * blk, :])

            def z4_bot_rep(y2, c1):
                return (z4rc[y2 // 8][:]
                        .rearrange("p (y c nn) -> p y c nn", y=8, c=4)
                        [:, y2 % 8, c1 - 4 * blk, :])

            wt = {}
            for li in range(2, min(7, maxli + 1)):
                wcols = max(SLABS[(li, blk, a)][1] for a in range(4)
                            if (li, blk, a) in SLABS)
                wt[li] = wpool.tile([128, wcols], F16, tag=f"w{li}", name=f"wt{li}")
                for a in range(4):
                    if (li, blk, a) not in SLABS:
                        continue
                    start, ncol = SLABS[(li, blk, a)]
                    if ncol == 0:
                        continue
                    getattr(nc, wq).dma_start(
                        wt[li][32 * a:32 * a + 16, 0:ncol],
                        wh[li][32 * a:32 * a + 16, start:start + ncol])

            for li, locs in schedule_blk(blk):
                if li > maxli:
                    continue
                lgets, rgets = [], []
                for (y, x) in locs:
                    if li == 2:
                        lgets.append(lambda yy=y, xx=x: z4_top(2 * yy, xx))
                        rgets.append(lambda yy=y, xx=x: z4_bot_rep(yy, xx))
                    else:
                        orient = SPECS[li][4]
                        cl = (li - 1, y, 2 * x) if orient else (li - 1, 2 * y, x)
                        cr = (li - 1, y, 2 * x + 1) if orient else (li - 1, 2 * y + 1, x)
                        lgets.append(child_get(cl))
                        rgets.append(child_get(cr))
                emit_pair(li, locs, lgets, rgets, wt[li])
            flush_wave()       # z4 tiles recycle next blk

        for li_w in range(7, 11):
            if li_w > maxli:
                continue
            flush_wave()       # wpool tiles recycle per tail layer
            wcols = max(SLABS[(li_w, 0, a)][1] for a in range(4)
                        if (li_w, 0, a) in SLABS)
            wtg = wpool.tile([128, wcols], F16, tag="wtail", name=f"wtg{li_w}")
            for a in range(4):
                if (li_w, 0, a) not in SLABS:
                    continue
                start, ncol = SLABS[(li_w, 0, a)]
                if ncol == 0:
                    continue
                getattr(nc, wq).dma_start(
                    wtg[32 * a:32 * a + 16, 0:ncol],
                    wh[li_w][32 * a:32 * a + 16, start:start + ncol])
            for li, locs in schedule_tail():
                if li != li_w:
                    continue
                lgets, rgets = [], []
                for (y, x) in locs:
                    orient = SPECS[li][4]
                    cl = (li - 1, y, 2 * x) if orient else (li - 1, 2 * y, x)
                    cr = (li - 1, y, 2 * x + 1) if orient else (li - 1, 2 * y + 1, x)
                    lgets.append(child_get(cl))
                    rgets.append(child_get(cr))
                emit_pair(li, locs, lgets, rgets, wtg)

        flush_wave()
        if maxli >= 11:
            pt = ps_r.tile([128, 512], F32, tag="r", name="pt11")
            nc.tensor.matmul(pt[0:16, 0:256], w11t[32 * q11:32 * q11 + 16, :],
                             cpx[(10, 0, 0)], start=True, stop=True,
                             tile_position=(32 * q11, 0))
            m11 = mpool.tile([16, 256], F16, tag="m11x", name="m11")
            nc.vector.tensor_mul(m11[:], pt[0:16, 0:256], rep[(10, 0, 1)][0:16, :])
            pf = ps_c.tile([128, 512], F32, tag="cp", name="pf")
            nc.tensor.matmul(pf[0:1, 0:256], ones16[0:16, 0:1], m11[:],
                             start=True, stop=True, tile_position=(0, 0))
            osb = tpool.tile([1, 256], F32, tag="outs")
            nc.scalar.copy(osb[:], pf[0:1, 0:256])
            nc.sync.dma_start(out_h[:], osb[:])
        else:
            zz = tpool.tile([1, 256], F32, tag="outs", name="zz")
            nc.any.memset(zz[:], 0.0)
            nc.sync.dma_start(out_h[:], zz[:])
        if dbg is not None:
            dsb = tpool.tile([128, 256], F16, tag="dbgt", name="dbgt")
            if dbg in rep:
                nc.vector.tensor_copy(dsb[:], rep[dbg])
            else:
                nc.any.memset(dsb[:], 0.0)
                nc.vector.tensor_copy(dsb[0:16, :], cpx[dbg])
            nc.sync.dma_start(dbg_h[:], dsb[:])
    nc.compile()
    return nc


def _get_program():
    global _PROGRAM
    if _PROGRAM is None:
        _PROGRAM = build_program()
    return _PROGRAM


def make_inputs(x, ws, core):
    whbm = pack_weights(ws)
    base = {f"w{li}": whbm[li] for li in range(2, 11)}
    base["w11"] = whbm[11]
    base.update(build_consts())
    z4e, z4r = pack_z4(x[core * NSH:(core + 1) * NSH])
    base["z4e"], base["z4r"] = z4e, z4r
    return base


def kernel(**inputs):
    from concourse.bass_utils import run_bass_kernel_spmd
    x = np.asarray(inputs['x'])
    ws = [np.asarray(inputs[f'w{i}']) for i in range(12)]
    whbm = pack_weights(ws)
    consts = build_consts()
    nc = _get_program()
    base = {f"w{li}": whbm[li] for li in range(2, 11)}
    base["w11"] = whbm[11]
    base.update(consts)
    in_maps = []
    for core in range(NCORES):
        z4e, z4r = pack_z4(x[core * NSH:(core + 1) * NSH])
        m = dict(base)
        m["z4e"], m["z4r"] = z4e, z4r
        in_maps.append(m)
    res = run_bass_kernel_spmd(nc, in_maps, list(range(NCORES)))
    out = np.concatenate([res.results[c]["out"].reshape(NSH)
                          for c in range(NCORES)])
    return out.reshape(BATCH, 1, 1, 1).astype(np.float32)



# revision 18
# speedup vs baseline: 1.1110x; 1.0368x over previous
"""Trainium2 Bass kernel for nn_BinaryTTN (batch 2048, 12-layer binary tree
tensor network), data-parallel across 8 NeuronCores.

Structure (per core, n=256 samples):
  * Layers 0+1 folded on host into layer-2 weights; the 16 z4 monomials per
    2x2 patch are pure input packing and are computed on host (fp32, cast to
    fp16) and DMA'd in, in the layout the layer-2 matmuls consume.
  * Each tree location (li=2..10): t = Wfold^T l (PE, K=16 row-tiled, rhs is
    the left child's compact [16,n] output), then an elementwise multiply by
    the right child's REP-form [128,n] (8x replicated rows), split across
    DVE/ACT/GPSIMD paths, then a PE reduction over j:
      - locations consumed as LEFT by their parent reduce via a col-tiled
        [K=128, M=16] matmul into a shared compact PSUM bank (4 locs/bank),
        evacuated once per 4 locs -> compact [16,n] fp16.
      - locations consumed as RIGHT reduce via the M=128 replicating matmul
        (baseline 'red') -> REP form, evacuated per loc.
  * Multiply paths (per pair of locations, statically assigned):
      D: DVE reads t from PSUM fp32 directly (1x mode)
      A: ACT evacuates t to fp16 SBUF, DVE multiplies at 2x
      GD: DVE copies t PSUM->SBUF, GPSIMD multiplies
      GA: ACT evacuates, GPSIMD multiplies
"""
import sys
import numpy as np
import ml_dtypes

BF16 = ml_dtypes.bfloat16

sys.path.insert(0, '/opt/trn_rl_repo')

BATCH, EMBED, H0, W0 = 2048, 2, 64, 64
NCORES = 8
NSH = BATCH // NCORES      # 256
NBLK = 8

# mult-path pattern, cycled over loc-pairs:
#   S   = fused evac+mult on DVE (scalar_tensor_tensor from PSUM, 1x)
#   A   = ACT evac -> DVE dense per-chunk mults (2x)
#   AG  = ACT evac -> DVE chunk0 + GPSIMD chunk1
#   AGG = ACT evac -> GPSIMD both chunks
PATH_PATTERN = ['S', 'A', 'S', 'AG', 'S', 'AGG', 'S', 'A',
                'S', 'AG', 'S', 'AG', 'A', 'S', 'AG', 'S']


def layer_specs():
    out = []
    H, W, ind = H0, W0, EMBED
    for li in range(12):
        bond = 1 if li == 11 else 16
        o = H < W
        h = H // (1 if o else 2)
        w = W // (2 if o else 1)
        out.append((h, w, bond, ind, o))
        H, W, ind = h, w, bond
    return out


SPECS = layer_specs()

# z4 partition-group permutation per input column: makes the four t-matmuls
# of a wave land on four distinct PE row-groups (full tile concurrency)
SIG = [0, 2, 1, 3]


def role_of(li, y, x):
    """'l' or 'r': how the parent consumes this loc's output."""
    if li == 11:
        return 'l'
    o_p = SPECS[li + 1][4]
    if o_p:
        return 'l' if x % 2 == 0 else 'r'
    return 'l' if y % 2 == 0 else 'r'


def schedule_blk(blk):
    """Yields (li, [locs...]) pair-batches; four independent y2 rows are
    interleaved so the scheduler always has independent work nearby."""
    for y2 in range(0, 16, 4):
        for c0 in range(0, 4, 2):
            for dy in range(4):
                yield (2, [(y2 + dy, 4 * blk + c0), (y2 + dy, 4 * blk + c0 + 1)])
        for dy in range(4):
            yield (3, [(y2 + dy, 2 * blk), (y2 + dy, 2 * blk + 1)])
        y4 = y2 // 2
        yield (4, [(y4, 2 * blk), (y4, 2 * blk + 1)])
        yield (4, [(y4 + 1, 2 * blk), (y4 + 1, 2 * blk + 1)])
        yield (5, [(y4, blk)])
        yield (5, [(y4 + 1, blk)])
        if y2 == 4:
            yield (6, [(0, blk), (1, blk)])
        elif y2 == 12:
            yield (6, [(2, blk), (3, blk)])


def schedule_tail():
    for li in range(7, 11):
        h, w = SPECS[li][0], SPECS[li][1]
        locs = [(y, x) for y in range(h) for x in range(w)]
        for i0 in range(0, len(locs), 2):
            yield (li, locs[i0:i0 + 2])


def _build_structure():
    """Walks the schedule; assigns per-loc meta:
      role, a (t-MM row group = q of left child / g of z4 slice),
      q+group (l-locs), path (per pair), slab offset per (li, blk, a)."""
    meta = {}
    slab_off = {}
    slabs = {}

    def loc_children(li, y, x):
        orient = SPECS[li][4]
        cl = (li - 1, y, 2 * x) if orient else (li - 1, 2 * y, x)
        cr = (li - 1, y, 2 * x + 1) if orient else (li - 1, 2 * y + 1, x)
        return cl, cr

    # pass 1: emission order + consumer batch index per loc
    batches = []
    for blk in range(NBLK):
        for li, locs in schedule_blk(blk):
            batches.append((li, locs, blk))
        batches.append(None)            # group-flush boundary
    for li, locs in schedule_tail():
        batches.append((li, locs, 0))
    batches.append(None)

    consumed_at = {}
    for bi, b in enumerate(batches):
        if b is None:
            continue
        li, locs, blk = b
        for (y, x) in locs:
            if li >= 3:
                cl, cr = loc_children(li, y, x)
                consumed_at[cl] = bi
                consumed_at[cr] = bi

    # pass 2: consumer-aware group assignment
    state = {'group': 0, 'pair': 0}
    groups = {}
    open_members = []

    def close_group():
        if open_members:
            groups[state['group']] = len(open_members)
            state['group'] += 1
            open_members.clear()

    def emit(li, locs, blk, bi):
        path = PATH_PATTERN[state['pair'] % len(PATH_PATTERN)]
        state['pair'] += 1
        for (y, x) in locs:
            if li == 2:
                a = ((y % 4) + SIG[x % 4]) % 4
            else:
                cl, _ = loc_children(li, y, x)
                a = meta[cl]['q']
            off = slab_off.get((li, a), 0)
            slab_off[(li, a)] = off + 256
            if (li, blk, a) not in slabs:
                slabs[(li, blk, a)] = off      # start col for this blk's slab
            m = dict(role=role_of(li, y, x), a=a, blk=blk, path=path,
                     woff=off - slabs[(li, blk, a)])
            if m['role'] == 'l':
                m['q'] = (len(open_members) + 2 * (state['group'] % 2)) % 4
                m['group'] = state['group']
                open_members.append((li, y, x))
                if len(open_members) == 4:
                    close_group()
            meta[(li, y, x)] = m

    for bi, b in enumerate(batches):
        if b is None:
            close_group()
            continue
        li, locs, blk = b
        emit(li, locs, blk, bi)
        # close if the next batch consumes any open member
        nxt = batches[bi + 1] if bi + 1 < len(batches) else None
        if open_members and nxt is not None:
            if any(consumed_at.get(k, 10 ** 9) <= bi + 1 for k in open_members):
                close_group()

    # sanity: every loc's group must close before its consumer batch
    close_bi = {}
    cnt = {}
    for bi, b in enumerate(batches):
        if b is None:
            continue
        li, locs, blk = b
        for (y, x) in locs:
            m = meta[(li, y, x)]
            if 'group' in m:
                g = m['group']
                cnt[g] = cnt.get(g, 0) + 1
                if cnt[g] == groups[g]:
                    close_bi[g] = bi
    for (li, y, x), m in meta.items():
        if 'group' in m and (li, y, x) in consumed_at:
            assert close_bi[m['group']] < consumed_at[(li, y, x)], \
                f"group {m['group']} closes too late for {(li, y, x)}"

    slabsz = {}
    for (li, blk, a), start in slabs.items():
        end = slab_off[(li, a)]
        nxt = min((s for (li2, b2, a2), s in slabs.items()
                   if li2 == li and a2 == a and s > start), default=end)
        slabsz[(li, blk, a)] = (start, nxt - start)
    shapes = {li: max(slab_off.get((li, a), 0) for a in range(4))
              for li in range(2, 11)}
    return meta, slabsz, shapes, groups


LOC_META, SLABS, WSHAPES, GROUPS = _build_structure()


# ---------------- host packing ----------------
def fold_weights(ws):
    W0m = ws[0].reshape(32, 64, 16, 4)
    T1 = np.einsum('rcbij,rcik,rcjl->rcbkl',
                   ws[1], W0m[:, 0::2], W0m[:, 1::2]).reshape(32, 32, 16, 16)
    Q2 = np.einsum('rcbij,rcik,rcjl->rcbkl', ws[2], T1[0::2], T1[1::2])
    folded = {}
    q = Q2.transpose(0, 1, 3, 2, 4).reshape(16, 32, 16, 2, 8, 16)
    folded[2] = q.transpose(0, 1, 3, 2, 4, 5).reshape(16, 32, 2, 16, 128)
    for li in range(3, 11):
        h, w = SPECS[li][0], SPECS[li][1]
        q = ws[li].transpose(0, 1, 3, 2, 4).reshape(h, w, 16, 2, 8, 16)
        folded[li] = q.transpose(0, 1, 3, 2, 4, 5).reshape(h, w, 2, 16, 128)
    folded[11] = ws[11].transpose(0, 1, 3, 2, 4).reshape(16, 16)
    return folded


def pack_weights(ws):
    folded = fold_weights(ws)
    whbm = {}
    for li in range(2, 11):
        arr = np.zeros((128, WSHAPES[li]), dtype=BF16)
        for (lli, y, x), m in LOC_META.items():
            if lli != li:
                continue
            a = m['a']
            off = SLABS[(li, m['blk'], a)][0] + m['woff']
            arr[32 * a:32 * a + 16, off:off + 256] = \
                folded[li][y, x].transpose(1, 0, 2).reshape(16, 256)
        whbm[li] = arr
    whbm[11] = folded[11].astype(BF16)
    return whbm


def build_consts():
    c = {}
    red = np.zeros((128, 256), dtype=BF16)
    for ch in range(2):
        for bl in range(8):
            b = ch * 8 + bl
            for k in range(16):
                for s in range(8):
                    red[bl * 16 + k, ch * 128 + s * 16 + b] = 1.0
    c['red'] = red
    red16 = np.zeros((128, 32), dtype=BF16)
    for b in range(16):
        bl, ch = b % 8, b // 8
        for j in range(16):
            red16[bl * 16 + j, ch * 16 + b] = 1.0
    c['red16'] = red16
    repk = np.zeros((128, 128), dtype=BF16)
    for a in range(4):
        for k in range(16):
            for s in range(8):
                repk[32 * a + k, s * 16 + k] = 1.0
    c['repk'] = repk
    o = np.zeros((128, 4), dtype=BF16)
    o[0:16, 0] = 1.0
    c['ones16'] = o
    return c


def pack_z4(xsh):
    """xsh [n, 2, 64, 64] fp32 -> z4e/z4o [128, 8*4*4*n], z4r [128, 8*16*4*n].

    z4[n, k=(kl 4, kr 4), r1, c1] = z0L[kl] * z0R[kr], with
    z0[n, (i,j), r, c] = x[n,i,2r,c] * x[n,j,2r+1,c].
    z4e/z4o layout: [32g+k, (blk, rhi, cin, n)] = z4[n, k, rhi*8+2g+par, 4blk+cin]
    z4r layout: [16s+k, (blk, y2, cin, n)] = z4[n, k, 2*y2+1, 4blk+cin]
    """
    n = xsh.shape[0]
    x32 = np.asarray(xsh, dtype=np.float32)
    top = x32[:, :, 0::2, :]
    bot = x32[:, :, 1::2, :]
    z0 = np.einsum('nirc,njrc->nijrc', top, bot).reshape(n, 4, 32, 64)
    z4 = np.einsum('nkrc,nlrc->nklrc', z0[:, :, :, 0::2],
                   z0[:, :, :, 1::2]).reshape(n, 16, 32, 32).astype(BF16)
    # z4e/z4o: r1 = rhi*8 + g*2 + par
    z4v = z4.reshape(n, 16, 4, 4, 2, 8, 4)  # n, k, rhi, g, par, blk, cin
    arr = z4v[:, :, :, :, 0]  # even rows: n k rhi g blk cin
    arr = arr.transpose(3, 1, 4, 2, 5, 0)  # g k blk rhi cin n
    # permute partition group by SIG[cin] so each wave's four L2 t-matmuls
    # land on four distinct PE row groups: slice (g, cin) -> (g+SIG[cin]) % 4
    rot = np.empty_like(arr)
    for cin in range(4):
        rot[:, :, :, :, cin] = np.roll(arr[:, :, :, :, cin], SIG[cin], axis=0)
    arr = rot.reshape(4, 16, -1)
    full = np.zeros((4, 32, arr.shape[2]), dtype=BF16)
    full[:, 0:16] = arr
    z4e = np.ascontiguousarray(full.reshape(128, -1))
    # z4r: bottom rows r1 = 2*y2+1
    zb = z4[:, :, 1::2, :].reshape(n, 16, 16, 8, 4)  # n k y2 blk cin
    zb = zb.transpose(1, 3, 2, 4, 0)  # k blk y2 cin n
    zr = np.broadcast_to(zb[None], (8,) + zb.shape).reshape(128, -1)
    return z4e, np.ascontiguousarray(zr)


# ---------------- device program ----------------
_PROGRAM = None


def build_program(num_devices=NCORES, dbg=None, maxli=11, wq='sync'):
    from contextlib import ExitStack
    import concourse.bass as bass
    import concourse.tile as tile
    from concourse import bacc, mybir

    F16, F32 = mybir.dt.bfloat16, mybir.dt.float32
    n = NSH
    nc = bacc.Bacc("TRN2", target_bir_lowering=False, debug=False,
                   num_devices=num_devices)
    z4e_h = nc.declare_dram_parameter("z4e", [128, 8 * 4 * 4 * n], F16, isOutput=False)
    z4r_h = nc.declare_dram_parameter("z4r", [128, 8 * 16 * 4 * n], F16, isOutput=False)
    wh = {li: nc.declare_dram_parameter(f"w{li}", [128, WSHAPES[li]], F16,
                                        isOutput=False) for li in range(2, 11)}
    wh[11] = nc.declare_dram_parameter("w11", [16, 16], F16, isOutput=False)
    red_h = nc.declare_dram_parameter("red", [128, 256], F16, isOutput=False)
    red16_h = nc.declare_dram_parameter("red16", [128, 32], F16, isOutput=False)
    repk_h = nc.declare_dram_parameter("repk", [128, 128], F16, isOutput=False)
    ones_h = nc.declare_dram_parameter("ones16", [128, 4], F16, isOutput=False)
    out_h = nc.declare_dram_parameter("out", [1, n], F32, isOutput=True)
    dbg_h = (nc.declare_dram_parameter("dbg", [128, n], F16, isOutput=True)
             if dbg is not None else None)

    q11 = LOC_META[(10, 0, 0)]['q'] if (10, 0, 0) in LOC_META else 0

    with tile.TileContext(nc) as tc, ExitStack() as ctx:
        cpool = ctx.enter_context(tc.tile_pool(name="consts", bufs=1))
        red = cpool.tile([128, 256], F16); nc.sync.dma_start(red[:], red_h[:])
        red16 = cpool.tile([128, 32], F16); nc.sync.dma_start(red16[:], red16_h[:])
        repk = cpool.tile([128, 128], F16); nc.sync.dma_start(repk[:], repk_h[:])
        ones16 = cpool.tile([128, 4], F16); nc.sync.dma_start(ones16[:], ones_h[:])
        w11t = cpool.tile([128, 16], F16)
        nc.sync.dma_start(w11t[32 * q11:32 * q11 + 16, :], wh[11][:])

        z4pool = ctx.enter_context(tc.tile_pool(name="z4", bufs=2))
        z4rpool = ctx.enter_context(tc.tile_pool(name="z4r", bufs=1))
        wpool = ctx.enter_context(tc.tile_pool(name="w", bufs=2))
        tpool = ctx.enter_context(tc.tile_pool(name="t", bufs=6))
        mpool = ctx.enter_context(tc.tile_pool(name="m", bufs=6))
        csbpool = ctx.enter_context(tc.tile_pool(name="csb", bufs=28))
        reppool = ctx.enter_context(tc.tile_pool(name="rep", bufs=28))
        ps_t = ctx.enter_context(tc.tile_pool(name="ps_t", bufs=5, space="PSUM"))
        ps_c = ctx.enter_context(tc.tile_pool(name="ps_c", bufs=1, space="PSUM"))
        ps_r = ctx.enter_context(tc.tile_pool(name="ps_r", bufs=2, space="PSUM"))

        # HAM warmup: ~4us of dummy matmuls so the PE clock-gate opens
        # (K=8/8, 2.4 GHz) before the real work starts; they overlap the
        # initial z4/weight DMAs.
        warm_ps = ps_t.tile([128, 512], F32, tag="t", name="warm")
        for _ in range(36):
            nc.tensor.matmul(warm_ps[:, 0:128], red[:, 0:128], red[:, 0:128],
                             start=True, stop=True)

        rep = {}     # r-loc key -> [128, n] f16 AP
        uid = [0]
        cpx = {}     # l-loc key -> [16, n] f16 AP (slice of group tile)
        cpgrp = {'ps': None, 'members': [], 'gid': -1}

        def finish_group():
            st = cpgrp
            if st['ps'] is None:
                return
            csb = csbpool.tile([128, 256], F16, tag="csb")
            nc.scalar.copy(csb[:], st['ps'][:, 0:256])
            for (key, q) in st['members']:
                cpx[key] = csb[32 * q:32 * q + 16, :]
            st['ps'] = None
            st['members'] = []

        # ---- wave machinery: batch 2 pairs, emit PE work phase-major so
        # LDWEIGHTS overlaps in-flight matmuls (row/col-group rotation) ----
        wave = []        # entries: dict(li, locs, metas, lgets, rgets, wtile)
        wave_keys = set()

        def flush_wave():
            if not wave:
                return
            # phase 1: t-matmuls, chunk-major across locs -> row groups rotate
            for e in wave:
                tps = []
                for i in range(len(e['locs'])):
                    uid[0] += 1
                    tps.append(ps_t.tile([128, 512], F32, tag="t",
                                         name=f"tp{uid[0]}"))
                e['tps'] = tps
            for c in range(2):
                for e in wave:
                    for i in range(len(e['locs'])):
                        m = e['metas'][i]
                        a, woff = m['a'], m['woff']
                        nc.tensor.matmul(
                            e['tps'][i][:, c * 256:(c + 1) * 256],
                            e['wtile'][32 * a:32 * a + 16,
                                       woff + c * 128:woff + (c + 1) * 128],
                            e['lgets'][i](), start=True, stop=True,
                            tile_position=(32 * a, 0))
            # phase 2: multiplies
            for e in wave:
                nl = len(e['locs'])
                path = e['metas'][0]['path']
                msb = mpool.tile([128, 1024], F16, tag="m16")
                e['msb'] = msb
                if path == 'S':
                    for i in range(nl):
                        nc.vector.scalar_tensor_tensor(
                            msb[:, i * 512:(i + 1) * 512]
                            .rearrange("p (c nn) -> p c nn", c=2),
                            e['tps'][i][:].rearrange("p (c nn) -> p c nn", c=2),
                            1.0,
                            e['rgets'][i]().unsqueeze(1).broadcast_to([128, 2, n]),
                            op0=mybir.AluOpType.mult, op1=mybir.AluOpType.mult)
                else:
                    tsb = tpool.tile([128, 1024], F16, tag="t16")
                    for i in range(nl):
                        nc.scalar.copy(tsb[:, i * 512:(i + 1) * 512],
                                       e['tps'][i][:])
                    for i in range(nl):
                        r = e['rgets'][i]()
                        for c in range(2):
                            dst = msb[:, i * 512 + c * 256:i * 512 + (c + 1) * 256]
                            src = tsb[:, i * 512 + c * 256:i * 512 + (c + 1) * 256]
                            on_dve = path == 'A' or (path == 'AG' and c == 0)
                            (nc.vector if on_dve else nc.gpsimd).tensor_mul(
                                dst, src, r)
            # phase 3: reduces; l-jobs grouped by compact-group (col-rotated),
            # then r-jobs in sub-batches of 2 (ps_r depth)
            ljobs, rjobs = [], []
            for e in wave:
                for i, (y, x) in enumerate(e['locs']):
                    m = e['metas'][i]
                    (ljobs if m['role'] == 'l' else rjobs).append(
                        (e, i, (e['li'], y, x), m))
            # l: contiguous segments share a group id
            k0 = 0
            while k0 < len(ljobs):
                gid = ljobs[k0][3]['group']
                k1 = k0
                while k1 < len(ljobs) and ljobs[k1][3]['group'] == gid:
                    k1 += 1
                seg = ljobs[k0:k1]
                st = cpgrp
                if st['gid'] != gid:
                    finish_group()
                    uid[0] += 1
                    st['ps'] = ps_c.tile([128, 512], F32, tag="cp",
                                         name=f"cp{uid[0]}")
                    st['gid'] = gid
                for c in range(2):
                    for (e, i, key, m) in seg:
                        q = m['q']
                        nc.tensor.matmul(
                            st['ps'][32 * q:32 * q + 16, 0:256],
                            red16[:, 16 * c:16 * c + 16],
                            e['msb'][:, i * 512 + c * 256:i * 512 + (c + 1) * 256],
                            start=(c == 0), stop=(c == 1),
                            tile_position=(0, 32 * q))
                for (e, i, key, m) in seg:
                    st['members'].append((key, m['q']))
                if len(st['members']) == GROUPS[gid]:
                    finish_group()
                k0 = k1
            # r: sub-batches of 2
            for k0 in range(0, len(rjobs), 2):
                sub = rjobs[k0:k0 + 2]
                prs = []
                for _ in sub:
                    uid[0] += 1
                    prs.append(ps_r.tile([128, 512], F32, tag="r",
                                         name=f"pr{uid[0]}"))
                for c in range(2):
                    for (e, i, key, m), pr in zip(sub, prs):
                        nc.tensor.matmul(
                            pr[:, 0:256], red[:, c * 128:(c + 1) * 128],
                            e['msb'][:, i * 512 + c * 256:i * 512 + (c + 1) * 256],
                            start=(c == 0), stop=(c == 1))
                for (e, i, key, m), pr in zip(sub, prs):
                    uid[0] += 1
                    rsb = reppool.tile([128, 256], F16, tag="rep",
                                       name=f"rsb{uid[0]}")
                    nc.scalar.copy(rsb[:], pr[:, 0:256])
                    rep[key] = rsb[:]
            wave.clear()
            wave_keys.clear()

        def emit_pair(li, locs, lgets, rgets, wtile):
            orient = SPECS[li][4]
            if li >= 3:
                for (y, x) in locs:
                    cl = (li - 1, y, 2 * x) if orient else (li - 1, 2 * y, x)
                    cr = (li - 1, y, 2 * x + 1) if orient else (li - 1, 2 * y + 1, x)
                    if cl not in cpx or cr not in rep:
                        flush_wave()
                        break
            wave.append(dict(li=li, locs=locs,
                             metas=[LOC_META[(li, y, x)] for (y, x) in locs],
                             lgets=lgets, rgets=rgets, wtile=wtile))
            for (y, x) in locs:
                wave_keys.add((li, y, x))
            if len(wave) >= 2:
                flush_wave()

        def child_get(key):
            m = LOC_META[key]
            if m['role'] == 'l':
                return lambda k=key: cpx[k]
            return lambda k=key: rep[k]

        for blk in range(NBLK):
            z4c = z4pool.tile([128, 4 * 4 * n], F16, tag="z4e", name="z4ct")
            nc.sync.dma_start(
                z4c[:],
                z4e_h[:].rearrange("p (b f) -> p b f", b=8)[:, blk, :])
            z4rc = []
            for half in range(2):
                zr = z4rpool.tile([128, 8 * 4 * n], F16, tag=f"z4r{half}",
                                  name=f"z4r{half}")
                nc.sync.dma_start(
                    zr[:],
                    z4r_h[:].rearrange("p (b h2 f) -> p b h2 f", b=8, h2=2)
                    [:, blk, half, :])
                z4rc.append(zr)

            def z4_top(r1, c1):
                g, rhi = (((r1 & 7) // 2) + SIG[c1 % 4]) % 4, r1 >> 3
                return (z4c[32 * g:32 * g + 16, :]
                        .rearrange("p (rhi c nn) -> p rhi c nn", rhi=4, c=4)
                        [:, rhi, c1 - 4 * blk, :])

            def z4_bot_rep(y2, c1):
                return (z4rc[y2 // 8][:]
                        .rearrange("p (y c nn) -> p y c nn", y=8, c=4)
                        [:, y2 % 8, c1 - 4 * blk, :])

            wt = {}
            for li in range(2, min(7, maxli + 1)):
                wcols = max(SLABS[(li, blk, a)][1] for a in range(4)
                            if (li, blk, a) in SLABS)
                wt[li] = wpool.tile([128, wcols], F16, tag=f"w{li}", name=f"wt{li}")
                for a in range(4):
                    if (li, blk, a) not in SLABS:
                        continue
                    start, ncol = SLABS[(li, blk, a)]
                    if ncol == 0:
                        continue
                    getattr(nc, wq).dma_start(
                        wt[li][32 * a:32 * a + 16, 0:ncol],
                        wh[li][32 * a:32 * a + 16, start:start + ncol])

            for li, locs in schedule_blk(blk):
                if li > maxli:
                    continue
                lgets, rgets = [], []
                for (y, x) in locs:
                    if li == 2:
                        lgets.append(lambda yy=y, xx=x: z4_top(2 * yy, xx))
                        rgets.append(lambda yy=y, xx=x: z4_bot_rep(yy, xx))
                    else:
                        orient = SPECS[li][4]
                        cl = (li - 1, y, 2 * x) if orient else (li - 1, 2 * y, x)
                        cr = (li - 1, y, 2 * x + 1) if orient else (li - 1, 2 * y + 1, x)
                        lgets.append(child_get(cl))
                        rgets.append(child_get(cr))
                emit_pair(li, locs, lgets, rgets, wt[li])
            flush_wave()       # z4 tiles recycle next blk

        for li_w in range(7, 11):
            if li_w > maxli:
                continue
            flush_wave()       # wpool tiles recycle per tail layer
            wcols = max(SLABS[(li_w, 0, a)][1] for a in range(4)
                        if (li_w, 0, a) in SLABS)
            wtg = wpool.tile([128, wcols], F16, tag="wtail", name=f"wtg{li_w}")
            for a in range(4):
                if (li_w, 0, a) not in SLABS:
                    continue
                start, ncol = SLABS[(li_w, 0, a)]
                if ncol == 0:
                    continue
                getattr(nc, wq).dma_start(
                    wtg[32 * a:32 * a + 16, 0:ncol],
                    wh[li_w][32 * a:32 * a + 16, start:start + ncol])
            for li, locs in schedule_tail():
                if li != li_w:
                    continue
                lgets, rgets = [], []
                for (y, x) in locs:
                    orient = SPECS[li][4]
                    cl = (li - 1, y, 2 * x) if orient else (li - 1, 2 * y, x)
                    cr = (li - 1, y, 2 * x + 1) if orient else (li - 1, 2 * y + 1, x)
                    lgets.append(child_get(cl))
                    rgets.append(child_get(cr))
                emit_pair(li, locs, lgets, rgets, wtg)

        flush_wave()
        if maxli >= 11:
            pt = ps_r.tile([128, 512], F32, tag="r", name="pt11")
            nc.tensor.matmul(pt[0:16, 0:256], w11t[32 * q11:32 * q11 + 16, :],
                             cpx[(10, 0, 0)], start=True, stop=True,
                             tile_position=(32 * q11, 0))
            m11 = mpool.tile([16, 256], F16, tag="m11x", name="m11")
            nc.vector.tensor_mul(m11[:], pt[0:16, 0:256], rep[(10, 0, 1)][0:16, :])
            pf = ps_c.tile([128, 512], F32, tag="cp", name="pf")
            nc.tensor.matmul(pf[0:1, 0:256], ones16[0:16, 0:1], m11[:],
                             start=True, stop=True, tile_position=(0, 0))
            osb = tpool.tile([1, 256], F32, tag="outs")
            nc.scalar.copy(osb[:], pf[0:1, 0:256])
            nc.sync.dma_start(out_h[:], osb[:])
        else:
            zz = tpool.tile([1, 256], F32, tag="outs", name="zz")
            nc.any.memset(zz[:], 0.0)
            nc.sync.dma_start(out_h[:], zz[:])
        if dbg is not None:
            dsb = tpool.tile([128, 256], F16, tag="dbgt", name="dbgt")
            if dbg in rep:
                nc.vector.tensor_copy(dsb[:], rep[dbg])
            else:
                nc.any.memset(dsb[:], 0.0)
                nc.vector.tensor_copy(dsb[0:16, :], cpx[dbg])
            nc.sync.dma_start(dbg_h[:], dsb[:])
    nc.compile()
    return nc


def _get_program():
    global _PROGRAM
    if _PROGRAM is None:
        _PROGRAM = build_program()
    return _PROGRAM


def make_inputs(x, ws, core):
    whbm = pack_weights(ws)
    base = {f"w{li}": whbm[li] for li in range(2, 11)}
    base["w11"] = whbm[11]
    base.update(build_consts())
    z4e, z4r = pack_z4(x[core * NSH:(core + 1) * NSH])
    base["z4e"], base["z4r"] = z4e, z4r
    return base


def kernel(**inputs):
    from concourse.bass_utils import run_bass_kernel_spmd
    x = np.asarray(inputs['x'])
    ws = [np.asarray(inputs[f'w{i}']) for i in range(12)]
    whbm = pack_weights(ws)
    consts = build_consts()
    nc = _get_program()
    base = {f"w{li}": whbm[li] for li in range(2, 11)}
    base["w11"] = whbm[11]
    base.update(consts)
    in_maps = []
    for core in range(NCORES):
        z4e, z4r = pack_z4(x[core * NSH:(core + 1) * NSH])
        m = dict(base)
        m["z4e"], m["z4r"] = z4e, z4r
        in_maps.append(m)
    res = run_bass_kernel_spmd(nc, in_maps, list(range(NCORES)))
    out = np.concatenate([res.results[c]["out"].reshape(NSH)
                          for c in range(NCORES)])
    return out.reshape(BATCH, 1, 1, 1).astype(np.float32)



# revision 25
# speedup vs baseline: 1.1871x; 1.0685x over previous
"""Trainium2 Bass kernel for nn_BinaryTTN (batch 2048, 12-layer binary tree
tensor network), data-parallel across 8 NeuronCores.

Structure (per core, n=256 samples):
  * Layers 0+1 folded on host into layer-2 weights; the 16 z4 monomials per
    2x2 patch are pure input packing and are computed on host (fp32, cast to
    fp16) and DMA'd in, in the layout the layer-2 matmuls consume.
  * Each tree location (li=2..10): t = Wfold^T l (PE, K=16 row-tiled, rhs is
    the left child's compact [16,n] output), then an elementwise multiply by
    the right child's REP-form [128,n] (8x replicated rows), split across
    DVE/ACT/GPSIMD paths, then a PE reduction over j:
      - locations consumed as LEFT by their parent reduce via a col-tiled
        [K=128, M=16] matmul into a shared compact PSUM bank (4 locs/bank),
        evacuated once per 4 locs -> compact [16,n] fp16.
      - locations consumed as RIGHT reduce via the M=128 replicating matmul
        (baseline 'red') -> REP form, evacuated per loc.
  * Multiply paths (per pair of locations, statically assigned):
      D: DVE reads t from PSUM fp32 directly (1x mode)
      A: ACT evacuates t to fp16 SBUF, DVE multiplies at 2x
      GD: DVE copies t PSUM->SBUF, GPSIMD multiplies
      GA: ACT evacuates, GPSIMD multiplies
"""
import sys
import numpy as np
import ml_dtypes

BF16 = ml_dtypes.bfloat16

sys.path.insert(0, '/opt/trn_rl_repo')

BATCH, EMBED, H0, W0 = 2048, 2, 64, 64
NCORES = 8
NSH = BATCH // NCORES      # 256
NBLK = 8

# mult-path pattern, cycled over loc-pairs:
#   S   = fused evac+mult on DVE (scalar_tensor_tensor from PSUM, 1x)
#   A   = ACT evac -> DVE dense per-chunk mults (2x)
#   AG  = ACT evac -> DVE chunk0 + GPSIMD chunk1
#   AGG = ACT evac -> GPSIMD both chunks
PATH_PATTERN = ['S', 'AG', 'S', 'A', 'S', 'AGG', 'S', 'AG',
                'S', 'AG', 'S', 'AGG', 'S', 'AG', 'S', 'AG']


def layer_specs():
    out = []
    H, W, ind = H0, W0, EMBED
    for li in range(12):
        bond = 1 if li == 11 else 16
        o = H < W
        h = H // (1 if o else 2)
        w = W // (2 if o else 1)
        out.append((h, w, bond, ind, o))
        H, W, ind = h, w, bond
    return out


SPECS = layer_specs()

# z4 partition-group permutation per input column: makes the four t-matmuls
# of a wave land on four distinct PE row-groups (full tile concurrency)
SIG = [0, 2, 1, 3]


def role_of(li, y, x):
    """'l' or 'r': how the parent consumes this loc's output."""
    if li == 11:
        return 'l'
    o_p = SPECS[li + 1][4]
    if o_p:
        return 'l' if x % 2 == 0 else 'r'
    return 'l' if y % 2 == 0 else 'r'


def schedule_blk(blk):
    """Yields (li, [locs...]) pair-batches; four independent y2 rows are
    interleaved so the scheduler always has independent work nearby."""
    for y2 in range(0, 16, 4):
        for c0 in range(0, 4, 2):
            for dy in range(4):
                yield (2, [(y2 + dy, 4 * blk + c0), (y2 + dy, 4 * blk + c0 + 1)])
        for dy in range(4):
            yield (3, [(y2 + dy, 2 * blk), (y2 + dy, 2 * blk + 1)])
        y4 = y2 // 2
        yield (4, [(y4, 2 * blk), (y4, 2 * blk + 1)])
        yield (4, [(y4 + 1, 2 * blk), (y4 + 1, 2 * blk + 1)])
        yield (5, [(y4, blk)])
        yield (5, [(y4 + 1, blk)])
        if y2 == 4:
            yield (6, [(0, blk), (1, blk)])
        elif y2 == 12:
            yield (6, [(2, blk), (3, blk)])


def schedule_tail():
    for li in range(7, 11):
        h, w = SPECS[li][0], SPECS[li][1]
        locs = [(y, x) for y in range(h) for x in range(w)]
        for i0 in range(0, len(locs), 2):
            yield (li, locs[i0:i0 + 2])


def _build_structure():
    """Walks the schedule; assigns per-loc meta:
      role, a (t-MM row group = q of left child / g of z4 slice),
      q+group (l-locs), path (per pair), slab offset per (li, blk, a)."""
    meta = {}
    slab_off = {}
    slabs = {}

    def loc_children(li, y, x):
        orient = SPECS[li][4]
        cl = (li - 1, y, 2 * x) if orient else (li - 1, 2 * y, x)
        cr = (li - 1, y, 2 * x + 1) if orient else (li - 1, 2 * y + 1, x)
        return cl, cr

    # pass 1: emission order + consumer batch index per loc
    batches = []
    for blk in range(NBLK):
        for li, locs in schedule_blk(blk):
            batches.append((li, locs, blk))
        batches.append(None)            # group-flush boundary
    for li, locs in schedule_tail():
        batches.append((li, locs, 0))
    batches.append(None)

    consumed_at = {}
    for bi, b in enumerate(batches):
        if b is None:
            continue
        li, locs, blk = b
        for (y, x) in locs:
            if li >= 3:
                cl, cr = loc_children(li, y, x)
                consumed_at[cl] = bi
                consumed_at[cr] = bi

    # pass 2: consumer-aware group assignment
    state = {'group': 0, 'pair': 0}
    groups = {}
    open_members = []

    def close_group():
        if open_members:
            groups[state['group']] = len(open_members)
            state['group'] += 1
            open_members.clear()

    def emit(li, locs, blk, bi):
        path = PATH_PATTERN[state['pair'] % len(PATH_PATTERN)]
        state['pair'] += 1
        for (y, x) in locs:
            if li == 2:
                a = ((y % 4) + SIG[x % 4]) % 4
            else:
                cl, _ = loc_children(li, y, x)
                a = meta[cl]['q']
            off = slab_off.get((li, a), 0)
            slab_off[(li, a)] = off + 256
            if (li, blk, a) not in slabs:
                slabs[(li, blk, a)] = off      # start col for this blk's slab
            m = dict(role=role_of(li, y, x), a=a, blk=blk, path=path,
                     woff=off - slabs[(li, blk, a)])
            if m['role'] == 'l':
                m['q'] = (len(open_members) + 2 * (state['group'] % 2)) % 4
                m['group'] = state['group']
                open_members.append((li, y, x))
                if len(open_members) == 4:
                    close_group()
            meta[(li, y, x)] = m

    for bi, b in enumerate(batches):
        if b is None:
            close_group()
            continue
        li, locs, blk = b
        emit(li, locs, blk, bi)
        # close if the next batch consumes any open member
        nxt = batches[bi + 1] if bi + 1 < len(batches) else None
        if open_members and nxt is not None:
            if any(consumed_at.get(k, 10 ** 9) <= bi + 1 for k in open_members):
                close_group()

    # sanity: every loc's group must close before its consumer batch
    close_bi = {}
    cnt = {}
    for bi, b in enumerate(batches):
        if b is None:
            continue
        li, locs, blk = b
        for (y, x) in locs:
            m = meta[(li, y, x)]
            if 'group' in m:
                g = m['group']
                cnt[g] = cnt.get(g, 0) + 1
                if cnt[g] == groups[g]:
                    close_bi[g] = bi
    for (li, y, x), m in meta.items():
        if 'group' in m and (li, y, x) in consumed_at:
            assert close_bi[m['group']] < consumed_at[(li, y, x)], \
                f"group {m['group']} closes too late for {(li, y, x)}"

    slabsz = {}
    for (li, blk, a), start in slabs.items():
        end = slab_off[(li, a)]
        nxt = min((s for (li2, b2, a2), s in slabs.items()
                   if li2 == li and a2 == a and s > start), default=end)
        slabsz[(li, blk, a)] = (start, nxt - start)
    shapes = {li: max(slab_off.get((li, a), 0) for a in range(4))
              for li in range(2, 11)}
    return meta, slabsz, shapes, groups


LOC_META, SLABS, WSHAPES, GROUPS = _build_structure()


# ---------------- host packing ----------------
def fold_weights(ws):
    W0m = ws[0].reshape(32, 64, 16, 4)
    T1 = np.einsum('rcbij,rcik,rcjl->rcbkl',
                   ws[1], W0m[:, 0::2], W0m[:, 1::2]).reshape(32, 32, 16, 16)
    Q2 = np.einsum('rcbij,rcik,rcjl->rcbkl', ws[2], T1[0::2], T1[1::2])
    folded = {}
    q = Q2.transpose(0, 1, 3, 2, 4).reshape(16, 32, 16, 2, 8, 16)
    folded[2] = q.transpose(0, 1, 3, 2, 4, 5).reshape(16, 32, 2, 16, 128)
    for li in range(3, 11):
        h, w = SPECS[li][0], SPECS[li][1]
        q = ws[li].transpose(0, 1, 3, 2, 4).reshape(h, w, 16, 2, 8, 16)
        folded[li] = q.transpose(0, 1, 3, 2, 4, 5).reshape(h, w, 2, 16, 128)
    folded[11] = ws[11].transpose(0, 1, 3, 2, 4).reshape(16, 16)
    return folded


def pack_weights(ws):
    folded = fold_weights(ws)
    whbm = {}
    for li in range(2, 11):
        arr = np.zeros((128, WSHAPES[li]), dtype=BF16)
        for (lli, y, x), m in LOC_META.items():
            if lli != li:
                continue
            a = m['a']
            off = SLABS[(li, m['blk'], a)][0] + m['woff']
            arr[32 * a:32 * a + 16, off:off + 256] = \
                folded[li][y, x].transpose(1, 0, 2).reshape(16, 256)
        whbm[li] = arr
    whbm[11] = folded[11].astype(BF16)
    return whbm


def build_consts():
    c = {}
    red = np.zeros((128, 256), dtype=BF16)
    for ch in range(2):
        for bl in range(8):
            b = ch * 8 + bl
            for k in range(16):
                for s in range(8):
                    red[bl * 16 + k, ch * 128 + s * 16 + b] = 1.0
    c['red'] = red
    red16 = np.zeros((128, 32), dtype=BF16)
    for b in range(16):
        bl, ch = b % 8, b // 8
        for j in range(16):
            red16[bl * 16 + j, ch * 16 + b] = 1.0
    c['red16'] = red16
    repk = np.zeros((128, 128), dtype=BF16)
    for a in range(4):
        for k in range(16):
            for s in range(8):
                repk[32 * a + k, s * 16 + k] = 1.0
    c['repk'] = repk
    o = np.zeros((128, 4), dtype=BF16)
    o[0:16, 0] = 1.0
    c['ones16'] = o
    return c


def pack_z4(xsh):
    """xsh [n, 2, 64, 64] fp32 -> z4e/z4o [128, 8*4*4*n], z4r [128, 8*16*4*n].

    z4[n, k=(kl 4, kr 4), r1, c1] = z0L[kl] * z0R[kr], with
    z0[n, (i,j), r, c] = x[n,i,2r,c] * x[n,j,2r+1,c].
    z4e/z4o layout: [32g+k, (blk, rhi, cin, n)] = z4[n, k, rhi*8+2g+par, 4blk+cin]
    z4r layout: [16s+k, (blk, y2, cin, n)] = z4[n, k, 2*y2+1, 4blk+cin]
    """
    n = xsh.shape[0]
    x32 = np.asarray(xsh, dtype=np.float32)
    top = x32[:, :, 0::2, :]
    bot = x32[:, :, 1::2, :]
    z0 = np.einsum('nirc,njrc->nijrc', top, bot).reshape(n, 4, 32, 64)
    z4 = np.einsum('nkrc,nlrc->nklrc', z0[:, :, :, 0::2],
                   z0[:, :, :, 1::2]).reshape(n, 16, 32, 32).astype(BF16)
    # z4e/z4o: r1 = rhi*8 + g*2 + par
    z4v = z4.reshape(n, 16, 4, 4, 2, 8, 4)  # n, k, rhi, g, par, blk, cin
    arr = z4v[:, :, :, :, 0]  # even rows: n k rhi g blk cin
    arr = arr.transpose(3, 1, 4, 2, 5, 0)  # g k blk rhi cin n
    # permute partition group by SIG[cin] so each wave's four L2 t-matmuls
    # land on four distinct PE row groups: slice (g, cin) -> (g+SIG[cin]) % 4
    rot = np.empty_like(arr)
    for cin in range(4):
        rot[:, :, :, :, cin] = np.roll(arr[:, :, :, :, cin], SIG[cin], axis=0)
    arr = rot.reshape(4, 16, -1)
    full = np.zeros((4, 32, arr.shape[2]), dtype=BF16)
    full[:, 0:16] = arr
    z4e = np.ascontiguousarray(full.reshape(128, -1))
    # z4r: bottom rows r1 = 2*y2+1
    zb = z4[:, :, 1::2, :].reshape(n, 16, 16, 8, 4)  # n k y2 blk cin
    zb = zb.transpose(1, 3, 2, 4, 0)  # k blk y2 cin n
    zr = np.broadcast_to(zb[None], (8,) + zb.shape).reshape(128, -1)
    return z4e, np.ascontiguousarray(zr)


# ---------------- device program ----------------
_PROGRAM = None


def build_program(num_devices=NCORES, dbg=None, maxli=11, wq='sync'):
    from contextlib import ExitStack
    import concourse.bass as bass
    import concourse.tile as tile
    from concourse import bacc, mybir

    F16, F32 = mybir.dt.bfloat16, mybir.dt.float32
    n = NSH
    nc = bacc.Bacc("TRN2", target_bir_lowering=False, debug=False,
                   num_devices=num_devices)
    z4e_h = nc.declare_dram_parameter("z4e", [128, 8 * 4 * 4 * n], F16, isOutput=False)
    z4r_h = nc.declare_dram_parameter("z4r", [128, 8 * 16 * 4 * n], F16, isOutput=False)
    wh = {li: nc.declare_dram_parameter(f"w{li}", [128, WSHAPES[li]], F16,
                                        isOutput=False) for li in range(2, 11)}
    wh[11] = nc.declare_dram_parameter("w11", [16, 16], F16, isOutput=False)
    red_h = nc.declare_dram_parameter("red", [128, 256], F16, isOutput=False)
    red16_h = nc.declare_dram_parameter("red16", [128, 32], F16, isOutput=False)
    repk_h = nc.declare_dram_parameter("repk", [128, 128], F16, isOutput=False)
    ones_h = nc.declare_dram_parameter("ones16", [128, 4], F16, isOutput=False)
    out_h = nc.declare_dram_parameter("out", [1, n], F32, isOutput=True)
    dbg_h = (nc.declare_dram_parameter("dbg", [128, n], F16, isOutput=True)
             if dbg is not None else None)

    q11 = LOC_META[(10, 0, 0)]['q'] if (10, 0, 0) in LOC_META else 0

    with tile.TileContext(nc) as tc, ExitStack() as ctx:
        cpool = ctx.enter_context(tc.tile_pool(name="consts", bufs=1))
        red = cpool.tile([128, 256], F16); nc.sync.dma_start(red[:], red_h[:])
        red16 = cpool.tile([128, 32], F16); nc.sync.dma_start(red16[:], red16_h[:])
        repk = cpool.tile([128, 128], F16); nc.sync.dma_start(repk[:], repk_h[:])
        ones16 = cpool.tile([128, 4], F16); nc.sync.dma_start(ones16[:], ones_h[:])
        w11t = cpool.tile([128, 16], F16)
        nc.sync.dma_start(w11t[32 * q11:32 * q11 + 16, :], wh[11][:])

        z4pool = ctx.enter_context(tc.tile_pool(name="z4", bufs=2))
        z4rpool = ctx.enter_context(tc.tile_pool(name="z4r", bufs=1))
        wpool = ctx.enter_context(tc.tile_pool(name="w", bufs=2))
        tpool = ctx.enter_context(tc.tile_pool(name="t", bufs=6))
        mpool = ctx.enter_context(tc.tile_pool(name="m", bufs=6))
        csbpool = ctx.enter_context(tc.tile_pool(name="csb", bufs=28))
        reppool = ctx.enter_context(tc.tile_pool(name="rep", bufs=28))
        ps_t = ctx.enter_context(tc.tile_pool(name="ps_t", bufs=5, space="PSUM"))
        ps_c = ctx.enter_context(tc.tile_pool(name="ps_c", bufs=1, space="PSUM"))
        ps_r = ctx.enter_context(tc.tile_pool(name="ps_r", bufs=2, space="PSUM"))

        # HAM warmup: ~4us of dummy matmuls so the PE clock-gate opens
        # (K=8/8, 2.4 GHz) before the real work starts; they overlap the
        # initial z4/weight DMAs.
        warm_ps = ps_t.tile([128, 512], F32, tag="t", name="warm")
        for _ in range(36):
            nc.tensor.matmul(warm_ps[:, 0:128], red[:, 0:128], red[:, 0:128],
                             start=True, stop=True)

        rep = {}     # r-loc key -> [128, n] f16 AP
        uid = [0]
        cpx = {}     # l-loc key -> [16, n] f16 AP (slice of group tile)
        cpgrp = {'ps': None, 'members': [], 'gid': -1}

        def finish_group():
            st = cpgrp
            if st['ps'] is None:
                return
            csb = csbpool.tile([128, 256], F16, tag="csb")
            nc.scalar.copy(csb[:], st['ps'][:, 0:256])
            for (key, q) in st['members']:
                cpx[key] = csb[32 * q:32 * q + 16, :]
            st['ps'] = None
            st['members'] = []

        # ---- wave machinery: batch 2 pairs, emit PE work phase-major so
        # LDWEIGHTS overlaps in-flight matmuls (row/col-group rotation).
        # Reduces are software-pipelined one wave: while the DVE multiplies
        # wave N, the PE already streams wave N+1's t-matmuls; wave N's
        # reduce matmuls are emitted at wave N+1's flush. ----
        wave = []        # entries: dict(li, locs, metas, lgets, rgets, wtile)
        wave_keys = set()
        pending_reduce = []

        def emit_tmms(entries):
            # t-matmuls, chunk-major across locs -> row groups rotate
            for e in entries:
                tps = []
                for i in range(len(e['locs'])):
                    uid[0] += 1
                    tps.append(ps_t.tile([128, 512], F32, tag="t",
                                         name=f"tp{uid[0]}"))
                e['tps'] = tps
            for c in range(2):
                for e in entries:
                    for i in range(len(e['locs'])):
                        m = e['metas'][i]
                        a, woff = m['a'], m['woff']
                        nc.tensor.matmul(
                            e['tps'][i][:, c * 256:(c + 1) * 256],
                            e['wtile'][32 * a:32 * a + 16,
                                       woff + c * 128:woff + (c + 1) * 128],
                            e['lgets'][i](), start=True, stop=True,
                            tile_position=(32 * a, 0))

        def emit_mults(entries):
            for e in entries:
                nl = len(e['locs'])
                path = e['metas'][0]['path']
                msb = mpool.tile([128, 1024], F16, tag="m16")
                e['msb'] = msb
                if path == 'S':
                    for i in range(nl):
                        nc.vector.scalar_tensor_tensor(
                            msb[:, i * 512:(i + 1) * 512]
                            .rearrange("p (c nn) -> p c nn", c=2),
                            e['tps'][i][:].rearrange("p (c nn) -> p c nn", c=2),
                            1.0,
                            e['rgets'][i]().unsqueeze(1).broadcast_to([128, 2, n]),
                            op0=mybir.AluOpType.mult, op1=mybir.AluOpType.mult)
                else:
                    tsb = tpool.tile([128, 1024], F16, tag="t16")
                    for i in range(nl):
                        nc.scalar.copy(tsb[:, i * 512:(i + 1) * 512],
                                       e['tps'][i][:])
                    for i in range(nl):
                        r = e['rgets'][i]()
                        for c in range(2):
                            dst = msb[:, i * 512 + c * 256:i * 512 + (c + 1) * 256]
                            src = tsb[:, i * 512 + c * 256:i * 512 + (c + 1) * 256]
                            on_dve = path == 'A' or (path == 'AG' and c == 0)
                            (nc.vector if on_dve else nc.gpsimd).tensor_mul(
                                dst, src, r)

        def emit_reduces(entries):
            # reduces; l-jobs grouped by compact-group (col-rotated),
            # then r-jobs in sub-batches of 2 (ps_r depth)
            ljobs, rjobs = [], []
            for e in entries:
                for i, (y, x) in enumerate(e['locs']):
                    m = e['metas'][i]
                    (ljobs if m['role'] == 'l' else rjobs).append(
                        (e, i, (e['li'], y, x), m))
            # l: contiguous segments share a group id
            k0 = 0
            while k0 < len(ljobs):
                gid = ljobs[k0][3]['group']
                k1 = k0
                while k1 < len(ljobs) and ljobs[k1][3]['group'] == gid:
                    k1 += 1
                seg = ljobs[k0:k1]
                st = cpgrp
                if st['gid'] != gid:
                    finish_group()
                    uid[0] += 1
                    st['ps'] = ps_c.tile([128, 512], F32, tag="cp",
                                         name=f"cp{uid[0]}")
                    st['gid'] = gid
                for c in range(2):
                    for (e, i, key, m) in seg:
                        q = m['q']
                        nc.tensor.matmul(
                            st['ps'][32 * q:32 * q + 16, 0:256],
                            red16[:, 16 * c:16 * c + 16],
                            e['msb'][:, i * 512 + c * 256:i * 512 + (c + 1) * 256],
                            start=(c == 0), stop=(c == 1),
                            tile_position=(0, 32 * q))
                for (e, i, key, m) in seg:
                    st['members'].append((key, m['q']))
                if len(st['members']) == GROUPS[gid]:
                    finish_group()
                k0 = k1
            # r: sub-batches of 2
            for k0 in range(0, len(rjobs), 2):
                sub = rjobs[k0:k0 + 2]
                prs = []
                for _ in sub:
                    uid[0] += 1
                    prs.append(ps_r.tile([128, 512], F32, tag="r",
                                         name=f"pr{uid[0]}"))
                for c in range(2):
                    for (e, i, key, m), pr in zip(sub, prs):
                        nc.tensor.matmul(
                            pr[:, 0:256], red[:, c * 128:(c + 1) * 128],
                            e['msb'][:, i * 512 + c * 256:i * 512 + (c + 1) * 256],
                            start=(c == 0), stop=(c == 1))
                for (e, i, key, m), pr in zip(sub, prs):
                    uid[0] += 1
                    rsb = reppool.tile([128, 256], F16, tag="rep",
                                       name=f"rsb{uid[0]}")
                    nc.scalar.copy(rsb[:], pr[:, 0:256])
                    rep[key] = rsb[:]

        def flush_wave(drain=False):
            if wave:
                emit_tmms(wave)
                emit_mults(wave)
                emit_reduces(pending_reduce)
                pending_reduce.clear()
                pending_reduce.extend(wave)
                wave.clear()
                wave_keys.clear()
            if drain and pending_reduce:
                emit_reduces(pending_reduce)
                pending_reduce.clear()

        def emit_pair(li, locs, lgets, rgets, wtile):
            orient = SPECS[li][4]
            if li >= 3:
                for (y, x) in locs:
                    cl = (li - 1, y, 2 * x) if orient else (li - 1, 2 * y, x)
                    cr = (li - 1, y, 2 * x + 1) if orient else (li - 1, 2 * y + 1, x)
                    if cl not in cpx or cr not in rep:
                        flush_wave(drain=True)
                        break
            wave.append(dict(li=li, locs=locs,
                             metas=[LOC_META[(li, y, x)] for (y, x) in locs],
                             lgets=lgets, rgets=rgets, wtile=wtile))
            for (y, x) in locs:
                wave_keys.add((li, y, x))
            if len(wave) >= 2:
                flush_wave()

        def child_get(key):
            m = LOC_META[key]
            if m['role'] == 'l':
                return lambda k=key: cpx[k]
            return lambda k=key: rep[k]

        for blk in range(NBLK):
            z4c = z4pool.tile([128, 4 * 4 * n], F16, tag="z4e", name="z4ct")
            nc.sync.dma_start(
                z4c[:],
                z4e_h[:].rearrange("p (b f) -> p b f", b=8)[:, blk, :])
            z4rc = []
            for half in range(2):
                zr = z4rpool.tile([128, 8 * 4 * n], F16, tag=f"z4r{half}",
                                  name=f"z4r{half}")
                nc.sync.dma_start(
                    zr[:],
                    z4r_h[:].rearrange("p (b h2 f) -> p b h2 f", b=8, h2=2)
                    [:, blk, half, :])
                z4rc.append(zr)

            def z4_top(r1, c1):
                g, rhi = (((r1 & 7) // 2) + SIG[c1 % 4]) % 4, r1 >> 3
                return (z4c[32 * g:32 * g + 16, :]
                        .rearrange("p (rhi c nn) -> p rhi c nn", rhi=4, c=4)
                        [:, rhi, c1 - 4 * blk, :])

            def z4_bot_rep(y2, c1):
                return (z4rc[y2 // 8][:]
                        .rearrange("p (y c nn) -> p y c nn", y=8, c=4)
                        [:, y2 % 8, c1 - 4 * blk, :])

            wt = {}
            for li in range(2, min(7, maxli + 1)):
                wcols = max(SLABS[(li, blk, a)][1] for a in range(4)
                            if (li, blk, a) in SLABS)
                wt[li] = wpool.tile([128, wcols], F16, tag=f"w{li}", name=f"wt{li}")
                for a in range(4):
                    if (li, blk, a) not in SLABS:
                        continue
                    start, ncol = SLABS[(li, blk, a)]
                    if ncol == 0:
                        continue
                    getattr(nc, wq).dma_start(
                        wt[li][32 * a:32 * a + 16, 0:ncol],
                        wh[li][32 * a:32 * a + 16, start:start + ncol])

            for li, locs in schedule_blk(blk):
                if li > maxli:
                    continue
                lgets, rgets = [], []
                for (y, x) in locs:
                    if li == 2:
                        lgets.append(lambda yy=y, xx=x: z4_top(2 * yy, xx))
                        rgets.append(lambda yy=y, xx=x: z4_bot_rep(yy, xx))
                    else:
                        orient = SPECS[li][4]
                        cl = (li - 1, y, 2 * x) if orient else (li - 1, 2 * y, x)
                        cr = (li - 1, y, 2 * x + 1) if orient else (li - 1, 2 * y + 1, x)
                        lgets.append(child_get(cl))
                        rgets.append(child_get(cr))
                emit_pair(li, locs, lgets, rgets, wt[li])
            flush_wave()       # z4 tiles recycle next blk

        for li_w in range(7, 11):
            if li_w > maxli:
                continue
            flush_wave()       # wpool tiles recycle per tail layer
            wcols = max(SLABS[(li_w, 0, a)][1] for a in range(4)
                        if (li_w, 0, a) in SLABS)
            wtg = wpool.tile([128, wcols], F16, tag="wtail", name=f"wtg{li_w}")
            for a in range(4):
                if (li_w, 0, a) not in SLABS:
                    continue
                start, ncol = SLABS[(li_w, 0, a)]
                if ncol == 0:
                    continue
                getattr(nc, wq).dma_start(
                    wtg[32 * a:32 * a + 16, 0:ncol],
                    wh[li_w][32 * a:32 * a + 16, start:start + ncol])
            for li, locs in schedule_tail():
                if li != li_w:
                    continue
                lgets, rgets = [], []
                for (y, x) in locs:
                    orient = SPECS[li][4]
                    cl = (li - 1, y, 2 * x) if orient else (li - 1, 2 * y, x)
                    cr = (li - 1, y, 2 * x + 1) if orient else (li - 1, 2 * y + 1, x)
                    lgets.append(child_get(cl))
                    rgets.append(child_get(cr))
                emit_pair(li, locs, lgets, rgets, wtg)

        flush_wave(drain=True)
        if maxli >= 11:
            pt = ps_r.tile([128, 512], F32, tag="r", name="pt11")
            nc.tensor.matmul(pt[0:16, 0:256], w11t[32 * q11:32 * q11 + 16, :],
                             cpx[(10, 0, 0)], start=True, stop=True,
                             tile_position=(32 * q11, 0))
            m11 = mpool.tile([16, 256], F16, tag="m11x", name="m11")
            nc.vector.tensor_mul(m11[:], pt[0:16, 0:256], rep[(10, 0, 1)][0:16, :])
            pf = ps_c.tile([128, 512], F32, tag="cp", name="pf")
            nc.tensor.matmul(pf[0:1, 0:256], ones16[0:16, 0:1], m11[:],
                             start=True, stop=True, tile_position=(0, 0))
            osb = tpool.tile([1, 256], F32, tag="outs")
            nc.scalar.copy(osb[:], pf[0:1, 0:256])
            nc.sync.dma_start(out_h[:], osb[:])
        else:
            zz = tpool.tile([1, 256], F32, tag="outs", name="zz")
            nc.any.memset(zz[:], 0.0)
            nc.sync.dma_start(out_h[:], zz[:])
        if dbg is not None:
            dsb = tpool.tile([128, 256], F16, tag="dbgt", name="dbgt")
            if dbg in rep:
                nc.vector.tensor_copy(dsb[:], rep[dbg])
            else:
                nc.any.memset(dsb[:], 0.0)
                nc.vector.tensor_copy(dsb[0:16, :], cpx[dbg])
            nc.sync.dma_start(dbg_h[:], dsb[:])
    nc.compile()
    return nc


def _get_program():
    global _PROGRAM
    if _PROGRAM is None:
        _PROGRAM = build_program()
    return _PROGRAM


def make_inputs(x, ws, core):
    whbm = pack_weights(ws)
    base = {f"w{li}": whbm[li] for li in range(2, 11)}
    base["w11"] = whbm[11]
    base.update(build_consts())
    z4e, z4r = pack_z4(x[core * NSH:(core + 1) * NSH])
    base["z4e"], base["z4r"] = z4e, z4r
    return base


def kernel(**inputs):
    from concourse.bass_utils import run_bass_kernel_spmd
    x = np.asarray(inputs['x'])
    ws = [np.asarray(inputs[f'w{i}']) for i in range(12)]
    whbm = pack_weights(ws)
    consts = build_consts()
    nc = _get_program()
    base = {f"w{li}": whbm[li] for li in range(2, 11)}
    base["w11"] = whbm[11]
    base.update(consts)
    in_maps = []
    for core in range(NCORES):
        z4e, z4r = pack_z4(x[core * NSH:(core + 1) * NSH])
        m = dict(base)
        m["z4e"], m["z4r"] = z4e, z4r
        in_maps.append(m)
    res = run_bass_kernel_spmd(nc, in_maps, list(range(NCORES)))
    out = np.concatenate([res.results[c]["out"].reshape(NSH)
                          for c in range(NCORES)])
    return out.reshape(BATCH, 1, 1, 1).astype(np.float32)

